# revision 48
# baseline (speedup 1.0000x reference)
"""Tensor-parallel (over GQA head groups) multi-head attention for 8 trn2 cores.

Each core owns 4 query heads + their shared kv head (one GQA group), the
matching 384 rows of wqkv and 256 columns of wo.  Every core computes a full
[S, D] partial of the output projection; the host sums the 8 partials.

v3: fp8e4 DoubleRow matmuls (0.5 PE cycles/row) for the qkv projection and
the output projection, with 3-term error compensation:

    A·B = A_hi·B_hi + (A_hi·B_lo + A_lo·B_hi)        (A_lo·B_lo dropped)

where X_hi = e4m3(X), X_lo = e4m3(X - X_hi).  The hi·hi term packs two
k-tiles per DoubleRow instruction; the cross term packs the two products of
ONE k-tile as the DoubleRow pair.  Net PE cost is 0.75x of one fp16 pass at
~0.2% relative error.  h/wqkv/wo are split on the host; the attention output
o2 is split on-device (ACT cast + DVE subtract).  Attention (scores, exp,
AV) stays fp16 exactly as v2.

Scale bookkeeping (e4m3 subnormal floor is 2^-6; weights must be scaled up):
  wq' rows = 32*wqkv_q | 32*wqkv_k | 64*wqkv_v   (no 1/sqrt(hd) in weights)
  rope tables carry 1/32  -> q01s/q23s/ks at true scale
  exp computed as exp(s/8 - 1): scale=1/8 is the scores 1/sqrt(hd),
    bias -1 keeps ex, the denominator and its f16 reciprocal in range
  v psum (64x) evacuated with activation scale 1/64 -> v at true scale
  v65 ones column = 1/16 -> normalized o2 tiles carry 16x (e4m3 range ~85)
  wo' = 64*wo; final psum is 1024x; the host divides the partials by 1024.
"""

import sys

if "/opt/trn_rl_repo" not in sys.path:
    sys.path.insert(0, "/opt/trn_rl_repo")

import ml_dtypes
import numpy as np

S = 2048
D = 2048
HD = 64
N_HEAD = 32
N_KV = 8
NCORES = 8
QH_PER_CORE = N_HEAD // NCORES  # 4
KV_SIZE = N_KV * HD  # 512
E4 = ml_dtypes.float8_e4m3
OUT_DESCALE = 1024.0  # 16 (o2 tiles) * 64 (wo)

_CACHE = {}


def _build_module(debug=False):
    from contextlib import ExitStack

    import concourse.mybir as mybir
    import concourse.tile as tile
    from concourse import bacc
    from concourse.bass import ds

    FP = mybir.dt.float32
    F16 = mybir.dt.float16
    F8 = mybir.dt.float8e4
    DR = mybir.MatmulPerfMode.DoubleRow
    EXP = mybir.ActivationFunctionType.Exp

    nc = bacc.Bacc(
        "TRN2",
        target_bir_lowering=False,
        debug=False,
        enable_asserts=False,
        num_devices=NCORES,
    )
    # register the exp bias constant (softmax shift, see emit_expav)
    _c = nc.alloc_sbuf_tensor("const-float32-neg1", [128, 1], FP)
    nc.gpsimd.memset(_c.ap(), -1.0)
    nc.const_aps.aps[(FP, -1.0)] = _c.ap()
    # 1/16 ones-column value for the softmax denominator accumulator
    _c16 = nc.alloc_sbuf_tensor("const-f16-inv16", [128, 1], F16)
    nc.gpsimd.memset(_c16.ap(), 0.0625)
    inv16 = _c16.ap()
    nc.all_engine_barrier()

    # [p, 2048*g + s] = e4m3(hidden)[s, 128*g + p], hi and lo halves
    hT_hi = nc.dram_tensor("hT_hi", [128, 16 * S], F8, kind="ExternalInput").ap()
    hT_lo = nc.dram_tensor("hT_lo", [128, 16 * S], F8, kind="ExternalInput").ap()
    # [p, 384*g + r] = wq'[r, 128*g + p]; r: 0-255 q, 256-319 k, 320-383 v
    wq_hi = nc.dram_tensor("wq_hi", [128, 16 * 384], F8, kind="ExternalInput").ap()
    wq_lo = nc.dram_tensor("wq_lo", [128, 16 * 384], F8, kind="ExternalInput").ap()
    # [p, 2048*u + e] = 64*wo[e, 256*core + 128*u + p]
    wo_hi = nc.dram_tensor("wo_hi", [128, 2 * 2048], F8, kind="ExternalInput").ap()
    wo_lo = nc.dram_tensor("wo_lo", [128, 2 * 2048], F8, kind="ExternalInput").ap()
    # rope tables, 1/32 scale: cols 0:2048 cos/32, 2048:4096 sin'/32
    rqq = nc.dram_tensor("rqq", [128, 2 * S], F16, kind="ExternalInput").ap()
    # cols 0:128 tri[p, f] = (p <= f); cols 128:256 identity[p, f] = (p == f)
    tri = nc.dram_tensor("tri", [128, 256], F16, kind="ExternalInput").ap()
    # f32 identity in rows 64-127 (rhs of the f32r v transposes)
    idf = nc.dram_tensor("idf", [128, 64], mybir.dt.float32r, kind="ExternalInput").ap()
    out = nc.dram_tensor("out", [S, D], F16, kind="ExternalOutput").ap()
    if debug:
        dbg = {
            name: nc.dram_tensor(f"dbg_{name}", shape, F16, kind="ExternalOutput").ap()
            for name, shape in (
                ("q01s", [128, S]),
                ("q23s", [128, S]),
                ("ks", [128, S]),
                ("v65", [128, 16 * 65]),
                ("o2a", [128, S]),
                ("o2b", [128, S]),
                ("ex0", [128, 8 * 1024]),
                ("sc0", [128, 8 * 1024]),
                ("po0", [65, 1024]),
                ("poc0", [65, 1024]),
                ("rbr0", [64, 1024]),
            )
        }

    with tile.TileContext(nc) as tc, ExitStack() as ctx:
        const = ctx.enter_context(tc.tile_pool(name="const", bufs=1))
        # [p, lo/hi, g%4, r]
        wqt = [
            const.tile([128, 2, 4, 384], F8, tag=f"wq{k}", name=f"wq_sb{k}")
            for k in range(4)
        ]

        def wq_hi_pair(g2, m):
            # hi weights for the k-tile pair (g2, g2+1); [128, 2, 128]
            k, lj = divmod(g2, 4)
            return wqt[k][:, 1, lj : lj + 2, 128 * m : 128 * m + 128]

        def wq_cross(g, m):
            # (lo, hi) weights of one k-tile; [128, 2, 128]
            k, lj = divmod(g, 4)
            return wqt[k][:, 0:2, lj, 128 * m : 128 * m + 128]

        F32R = mybir.dt.float32r
        # [p, lo/hi, u, e]
        wo8 = const.tile([128, 2, 2, 2048], F8, tag="wo")
        rqq_sb = const.tile([128, 4096], F16, tag="rqq")
        tri_sb = const.tile([128, 256], F16, tag="tri")
        idf_sb = const.tile([128, 64], FP, tag="idf")
        ident64 = idf_sb[64:128, :].bitcast(F32R)

        qsw = ctx.enter_context(tc.tile_pool(name="qsw", bufs=1))
        q01s = qsw.tile([128, S], F16, tag="q01s")
        q23s = qsw.tile([128, S], F16, tag="q23s")
        ks = qsw.tile([128, S], F16, tag="ks")
        v_sb = qsw.tile([128, 16 * 65], F16, tag="v")
        o2a = qsw.tile([128, S], F16, tag="o2a")
        o2b = qsw.tile([128, S], F16, tag="o2b")
        # [p, hi/lo, a/b, s] fp8 split of the (16x-scaled) attention output
        o28 = qsw.tile([128, 2, 2, S], F8, tag="o28")
        v65 = v_sb.rearrange("p (j c) -> p j c", c=65)

        # attention pools (live through the whole kernel)
        expool = ctx.enter_context(tc.tile_pool(name="ex", bufs=4))
        rspool = ctx.enter_context(tc.tile_pool(name="rs", bufs=2))
        rbpool = ctx.enter_context(tc.tile_pool(name="rb", bufs=2))
        nmpool = ctx.enter_context(tc.tile_pool(name="nm", bufs=2))
        # attention psum pools are created after quarter 0 releases its
        # 3-bank ring (right side of the arena; release is LIFO per side)
        phaseA = ExitStack()
        psS = psO = None

        # projection-phase pools (close before the out-projection opens)
        phaseP = ExitStack()
        hpool = phaseP.enter_context(tc.tile_pool(name="hp", bufs=24))
        qraw = phaseP.enter_context(tc.tile_pool(name="qraw", bufs=1))
        q01 = qraw.tile([128, S], F16, tag="q01")
        q23 = qraw.tile([128, S], F16, tag="q23")
        kv = qraw.tile([128, S], F16, tag="kv")  # rows 0:64 = k (rope input)
        vraw = qraw.tile([128, S], FP, tag="vraw")  # rows 64:128 = v, f32
        scpool = phaseP.enter_context(tc.tile_pool(name="sc", bufs=4))
        phase0 = ExitStack()
        psA0 = phase0.enter_context(tc.tile_pool(name="psA0", bufs=1, space="PSUM"))
        psA = None

        SWAP_MASK = [i ^ 1 for i in range(32)]

        def rope_quarter(dst, raw, p, costab, sintab, q, nm):
            # dst = raw * cos + pairswap(raw) * sin' on [0:p, 512q:512q+512]
            cs = ds(512 * q, 512)
            sw = scpool.tile([128, 512], F16, tag="sc", name=f"sw_{nm}{q}")
            nc.vector.stream_shuffle(sw[0:p, :], raw[0:p, cs], SWAP_MASK)
            t0 = scpool.tile([128, 512], F16, tag="sc", name=f"t0_{nm}{q}")
            nc.vector.tensor_mul(t0[0:p, :], raw[0:p, cs], costab[0:p, cs])
            nc.vector.tensor_mul(sw[0:p, :], sw[0:p, :], sintab[0:p, cs])
            nc.vector.tensor_add(dst[0:p, cs], t0[0:p, :], sw[0:p, :])

        # global DMA plan: one ordered stream of large transfers.  hT comes as
        # 2048-column quads ([128, 4, 512] strided, >=512B contiguous runs),
        # hi half before lo half so the hi*hi matmuls can start early.
        hT_hi_r = hT_hi.rearrange("p (g s) -> p g s", s=2048)
        hT_lo_r = hT_lo.rearrange("p (g s) -> p g s", s=2048)
        rq_r = rqq.rearrange("p (h s) -> p h s", s=2048)
        hq_tiles = {}

        def dma_hq(q, t, which, half=None, eng=None):
            # quad t of quarter q, which: 0 hi / 1 lo
            if half is None:
                gs, n = 4 * t, 4
            else:
                gs, n = 4 * t + 2 * half, 2
            tile = hq_tiles.get((q, t))
            if tile is None:
                tile = hpool.tile([128, 2, 4, 512], F8, tag="hc", name=f"hq_{q}_{t}")
                hq_tiles[(q, t)] = tile
            src_r = hT_hi_r if which == 0 else hT_lo_r
            (eng or nc.sync).dma_start(
                tile[:, which, gs - 4 * t : gs - 4 * t + n, :],
                src_r[:, gs : gs + n, ds(512 * q, 512)],
            )

        def hq_hi_pair(q, g2):
            k, lj = divmod(g2, 4)
            return hq_tiles[(q, k)][:, 0, lj : lj + 2, :]

        def hq_cross(q, g):
            k, lj = divmod(g, 4)
            return hq_tiles[(q, k)][:, 0:2, lj, :]

        def dma_wq(k, which, eng=None):
            # which: 0 lo / 1 hi
            src = (wq_lo if which == 0 else wq_hi)[:, ds(1536 * k, 1536)]
            (eng or nc.sync).dma_start(
                wqt[k][:, which], src.rearrange("p (g r) -> p g r", r=384)
            )

        rqsb_r = rqq_sb.rearrange("p (h s) -> p h s", s=2048)

        def dma_rq(q):
            nc.sync.dma_start(
                rqsb_r[:, :, ds(512 * q, 512)], rq_r[:, :, ds(512 * q, 512)]
            )

        def dma_wo(which):
            src = (wo_lo if which == 0 else wo_hi)
            nc.sync.dma_start(
                wo8[:, which], src.rearrange("p (u e) -> p u e", e=2048)
            )

        def dma_wq0h(half):
            # startup split: pairs (0,1) land before (2,3)
            src = wq_hi[:, ds(768 * half, 768)]
            nc.sync.dma_start(
                wqt[0][:, 1, 2 * half : 2 * half + 2, :],
                src.rearrange("p (g r) -> p g r", r=384),
            )

        # quarter-0 feed is split across the SP and ACT DGE queues (same-
        # queue transfers serialize in the DMA model; ACT is idle until the
        # first evacuation): hq-hi + wq-lo on SP, wq-hi + hq-lo on ACT, so
        # each quad's supply is ~1.3us/queue vs 1.9us of PE work.
        dma_plan = [
            ("wq0ha", lambda: dma_wq0h(0)),
            ("hq00ha", lambda: dma_hq(0, 0, 0, 0)),
            ("wq0hb", lambda: dma_wq0h(1)),
            ("hq00hb", lambda: dma_hq(0, 0, 0, 1)),
            ("hq00l", lambda: dma_hq(0, 0, 1, eng=nc.gpsimd)),
            ("wq0l", lambda: dma_wq(0, 0)),
            ("wq1h", lambda: dma_wq(1, 1, eng=nc.gpsimd)),
            ("hq01h", lambda: dma_hq(0, 1, 0)),
            ("hq01l", lambda: dma_hq(0, 1, 1, eng=nc.gpsimd)),
            ("wq1l", lambda: dma_wq(1, 0)),
            ("wq2h", lambda: dma_wq(2, 1, eng=nc.gpsimd)),
            ("hq02h", lambda: dma_hq(0, 2, 0)),
            ("hq02l", lambda: dma_hq(0, 2, 1, eng=nc.gpsimd)),
            ("wq2l", lambda: dma_wq(2, 0)),
            ("wq3h", lambda: dma_wq(3, 1, eng=nc.gpsimd)),
            ("hq03h", lambda: dma_hq(0, 3, 0)),
            ("hq03l", lambda: dma_hq(0, 3, 1, eng=nc.gpsimd)),
            ("wq3l", lambda: dma_wq(3, 0)),
            ("tri", lambda: (nc.sync.dma_start(tri_sb[:], tri),
                             nc.sync.dma_start(idf_sb[:].bitcast(F32R), idf))),
            ("rq0", lambda: dma_rq(0)),
            ("hq10", lambda: (dma_hq(1, 0, 0), dma_hq(1, 0, 1))),
            ("hq11", lambda: (dma_hq(1, 1, 0), dma_hq(1, 1, 1))),
            ("hq12", lambda: (dma_hq(1, 2, 0), dma_hq(1, 2, 1))),
            ("rq1", lambda: dma_rq(1)),
            ("hq13", lambda: (dma_hq(1, 3, 0), dma_hq(1, 3, 1))),
            ("hq20", lambda: (dma_hq(2, 0, 0), dma_hq(2, 0, 1))),
            ("wo0", lambda: dma_wo(1)),
            ("hq21", lambda: (dma_hq(2, 1, 0), dma_hq(2, 1, 1))),
            ("hq22", lambda: (dma_hq(2, 2, 0), dma_hq(2, 2, 1))),
            ("wo1", lambda: dma_wo(0)),
            ("rq2", lambda: dma_rq(2)),
            ("hq23", lambda: (dma_hq(2, 3, 0), dma_hq(2, 3, 1))),
            ("hq30", lambda: (dma_hq(3, 0, 0), dma_hq(3, 0, 1))),
            ("hq31", lambda: (dma_hq(3, 1, 0), dma_hq(3, 1, 1))),
            ("hq32", lambda: (dma_hq(3, 2, 0), dma_hq(3, 2, 1))),
            ("rq3", lambda: dma_rq(3)),
            ("hq33", lambda: (dma_hq(3, 3, 0), dma_hq(3, 3, 1))),
        ]
        plan_pos = {key: idx for idx, (key, _) in enumerate(dma_plan)}
        feed_state = {"next": 0}

        def feed_until(key):
            stop = plan_pos[key] + 1
            while feed_state["next"] < stop:
                dma_plan[feed_state["next"]][1]()
                feed_state["next"] += 1

        def proj_quad(q, t, ms, pts, started, last=()):
            # all fp8 matmuls of quad t for the m-passes in ms.
            # last: set of m whose final matmul gets stop=True.
            # jj/lg outer so the DMA pieces are consumed in arrival order.
            for jj in range(2):
                g2 = 4 * t + 2 * jj
                for m in ms:
                    nc.tensor.matmul(
                        pts[m][:], wq_hi_pair(g2, m), hq_hi_pair(q, g2),
                        start=(m not in started), stop=False, perf_mode=DR,
                    )
                    started.add(m)
            for lg in range(4):
                g = 4 * t + lg
                for m in ms:
                    nc.tensor.matmul(
                        pts[m][:], wq_cross(g, m), hq_cross(q, g),
                        start=False, stop=(m in last and lg == 3), perf_mode=DR,
                    )

        def emit_vtrans(q, vtp):
            # v transposes for quarter q: f32r through the projection psum
            # ring slot, then one f16 evacuation into v65
            vt = vtp.tile([128, 512], FP, tag=vtp_tag(vtp), name=f"vt_{q}")
            for jj in range(4):
                j = 4 * q + jj
                nc.tensor.transpose(
                    vt[:, ds(64 * jj, 64)].bitcast(F32R),
                    vraw[64:128, ds(128 * j, 128)].bitcast(F32R),
                    ident64,
                )
            nc.vector.tensor_copy(
                v65[:, 4 * q : 4 * q + 4, 0:64],
                vt[:, 0:256].rearrange("p (jj c) -> p jj c", c=64),
            )
            nc.vector.tensor_copy(
                v65[:, 4 * q : 4 * q + 4, 64:65],
                inv16[:, 0:1][:, None, :].to_broadcast([128, 4, 1]),
            )

        def vtp_tag(pool):
            return "p0" if pool is psA0 else "pj"

        def evac_kv(pts_m, q):
            cs = ds(512 * q, 512)
            nc.scalar.copy(kv[0:64, cs], pts_m[0:64, :])
            # v psum is 64x; descale to true scale on the way out
            nc.scalar.mul(vraw[64:128, cs].bitcast(F32R), pts_m[64:128, :], 1.0 / 64.0)

        def gen_proj_quarter0():
            # quarter 0 runs all three m-passes interleaved at the pace the
            # DMA stream can sustain (the front is inherently supply-bound)
            pts = [
                psA0.tile([128, 512], FP, tag=f"p{m}", name=f"pj0_{m}")
                for m in range(3)
            ]
            started = set()
            feed_until("hq00hb")
            for t in range(3):
                # hi*hi needs this quad's hi pieces; cross needs lo too
                feed_until(["wq0l", "wq1l", "wq2l"][t])
                proj_quad(0, t, (0, 1, 2), pts, started)
                feed_until(["wq3l", "tri", "rq0"][t])
                yield 1280
            # quad 3 m-serially, kv first: rope-k (which gates the first
            # attention scores) starts earliest
            cos_t, sin_t = rqq_sb[:, 0:2048], rqq_sb[:, 2048:4096]
            feed_until("wq3l")
            for m, dst, dsts, p, nm in (
                (2, kv, ks, 64, "k"),
                (0, q01, q01s, 128, "q01"),
                (1, q23, q23s, 128, "q23"),
            ):
                proj_quad(0, 3, (m,), pts, started, last={m})
                if m == 2:
                    evac_kv(pts[m], 0)
                else:
                    nc.scalar.copy(dst[:, 0:512], pts[m][:])
                rope_quarter(dsts, dst, p, cos_t, sin_t, 0, nm)
                if m == 2:
                    nc.gpsimd.dma_start(ks[64:128, 0:512], ks[0:64, 0:512])
                feed_until("hq10")
                yield 940
            emit_vtrans(0, psA0)
            yield 80
            feed_until("hq11")
            yield 120

        def gen_proj_quarter(q):
            feed_until(f"hq{q}1")
            pts = {}
            for m in (0, 1):
                pts[m] = psA.tile([128, 512], FP, tag="pj", name=f"pj_{q}_{m}")
            started = set()
            for t in range(4):
                if t >= 2:
                    feed_until(f"hq{q}{min(t + 1, 3)}")
                proj_quad(q, t, (0, 1), pts, started,
                          last=({0, 1} if t == 3 else ()))
                yield 1280
            cs = ds(512 * q, 512)
            nc.scalar.copy(q01[:, cs], pts[0][:])
            rope_quarter(q01s, q01, 128, rqq_sb[:, 0:2048], rqq_sb[:, 2048:4096], q, "q01")
            yield 300
            nc.scalar.copy(q23[:, cs], pts[1][:])
            rope_quarter(q23s, q23, 128, rqq_sb[:, 0:2048], rqq_sb[:, 2048:4096], q, "q23")
            yield 300
            pt2 = psA.tile([128, 512], FP, tag="pj", name=f"pj_{q}_2")
            started2 = set()
            for t in range(4):
                proj_quad(q, t, (2,), pts={2: pt2}, started=started2,
                          last=({2} if t == 3 else ()))
                # deep-prefetch the next quarter while the psum ring is
                # the only DMA consumer
                if q < 3:
                    feed_until(f"hq{q + 1}{min(t, 3)}")
                yield 640
            evac_kv(pt2, q)
            rope_quarter(ks, kv, 64, rqq_sb[:, 0:2048], rqq_sb[:, 2048:4096], q, "k")
            yield 300
            # duplicate rotated k at partitions 64-127 (odd heads' score
            # matmuls read lhsT/rhs both at base 64)
            nc.gpsimd.dma_start(ks[64:128, cs], ks[0:64, cs])
            emit_vtrans(q, psA)
            if q < 3:
                feed_until(f"hq{q + 1}1")
            yield 120

        def gen_attention_chunk(c):
            nj = 4 * c + 4
            for hp in range(2):
                po = psO.tile([65, 1024], FP, tag="po", name=f"po_{c}_{hp}")

                def emit_scores(j):
                    r = j - 4 * c  # >= 0 on diagonal blocks
                    off = 128 * r if r >= 0 else 0
                    ps = psS.tile([128, 1024], FP, tag="ps", name=f"ps_{c}_{hp}_{j}")
                    for hh in range(2):
                        h = 2 * hp + hh
                        qt = q01s if h < 2 else q23s
                        base = 64 * (h % 2)
                        nc.tensor.matmul(
                            ps[:, ds(512 * hh + off, 512 - off)],
                            ks[base : base + 64, ds(128 * j, 128)],
                            qt[base : base + 64, ds(512 * c + off, 512 - off)],
                        )
                    return ps, off, r >= 0

                def emit_expav(j, ps, off, diag):
                    # exp(s/8 - 2): 1/8 is the scores 1/sqrt(hd); softmax is
                    # shift-invariant and the bias keeps the f16 sums and the
                    # denominator reciprocal in range
                    ex = expool.tile([128, 1024], F16, tag="ex", name=f"ex_{c}_{hp}_{j}")
                    if not diag:
                        nc.scalar.activation(ex[:], ps[:], EXP, bias=-1.0, scale=0.125)
                    else:
                        w = 512 - off
                        psv = ps.rearrange("p (h w) -> p h w", w=512)[:, :, ds(off, w)]
                        exv = ex.rearrange("p (h w) -> p h w", w=512)[:, :, ds(off, w)]
                        nc.scalar.activation(exv, psv, EXP, bias=-1.0, scale=0.125)
                        exd = ex.rearrange("p (h w) -> p h w", w=512)[:, :, ds(off, 128)]
                        nc.vector.tensor_mul(
                            exd,
                            exd,
                            tri_sb[:, 0:128][:, None, :].to_broadcast([128, 2, 128]),
                        )
                    if debug and c == 0:
                        sl = ds(1024 * (4 * hp + j), 1024)
                        nc.sync.dma_start(dbg["ex0"][:, sl], ex[:])
                        sc16 = expool.tile(
                            [128, 1024], F16, tag="ex", name=f"scd_{hp}_{j}"
                        )
                        nc.vector.tensor_copy(sc16[:], ps[:])
                        nc.sync.dma_start(dbg["sc0"][:, sl], sc16[:])
                    for hh in range(2):
                        nc.tensor.matmul(
                            po[0:65, ds(512 * hh + off, 512 - off)],
                            v_sb[:, ds(65 * j, 65)],
                            ex[:, ds(512 * hh + off, 512 - off)],
                            start=(j == 0),
                            stop=(j == nj - 1),
                            skip_group_check=True,
                        )

                # one-j lookahead: scores(j+1) land on the PE between
                # scores(j) and av(j) so the exp never stalls the PE
                def jcost(j):
                    # PE ns of one scores OR av pair at this block's trim
                    r = j - 4 * c
                    off = 128 * r if r >= 0 else 0
                    return int((512 - off) * 0.833)

                prev = emit_scores(0)
                for j in range(1, nj):
                    cur = emit_scores(j)
                    emit_expav(j - 1, *prev)
                    prev = cur
                    yield jcost(j) + jcost(j - 1)
                emit_expav(nj - 1, *prev)
                yield jcost(nj - 1)
                # evacuate the accumulator so the bank frees for the next
                # head pair, then normalize: reciprocal of the sums row,
                # broadcast back into the evacuated po bank, then per half:
                # f16 o2 piece -> fp8 hi (ACT cast) + fp8 lo (DVE subtract)
                poc = rspool.tile([65, 1024], F16, tag="rs", name=f"poc_{c}_{hp}")
                if debug and c == 0 and hp == 0:
                    pod = rspool.tile([65, 1024], F16, tag="pod", name="pod")
                    nc.vector.tensor_copy(pod[:], po[:])
                    nc.sync.dma_start(dbg["po0"], pod[:])
                nc.scalar.copy(poc[:, 512:1024], po[:, 512:1024])
                nc.vector.tensor_copy(poc[:, 0:512], po[:, 0:512])
                rbr = rbpool.tile([64, 1024], F16, tag="rbr", name=f"rbr_{c}_{hp}")
                dsttile = o2a if hp == 0 else o2b
                nm = nmpool.tile([64, 512], F16, tag="nm", name=f"nm_{c}_{hp}")
                nm8 = nmpool.tile([64, 2, 512], F8, tag="nm8", name=f"nm8_{c}_{hp}")
                cs = ds(512 * c, 512)
                for half in (1, 0):
                    hs = ds(512 * half, 512)
                    nc.tensor.matmul(
                        po[0:64, hs], tri_sb[64:65, 64:128], poc[64:65, hs],
                        start=True, stop=True,
                    )
                    # the fp8 hi/lo split runs on the (otherwise idle) Pool
                    # engine, except the last chunk where ACT/DVE are free
                    # and Pool's ~1us/op latency would stretch the tail; the
                    # upper-half DMAs dispatch as soon as each piece exists
                    if c == 3:
                        cast_f = nc.scalar.copy
                        sub_e = nc.gpsimd if half == 1 else nc.vector
                    else:
                        cast_f, sub_e = nc.gpsimd.tensor_copy, nc.gpsimd
                    with nc.allow_low_precision(reason="softmax denom recip f16"):
                        nc.vector.reciprocal(rbr[0:64, hs], po[0:64, hs])
                        if half == 0:
                            piece = dsttile[0:64, cs]
                            nc.vector.tensor_mul(piece, poc[0:64, hs], rbr[0:64, hs])
                            hi8 = o28[0:64, 0, hp, cs]
                            cast_f(hi8, piece)
                            sub_e.tensor_sub(o28[0:64, 1, hp, cs], piece, hi8)
                        else:
                            nc.vector.tensor_mul(nm[0:64, :], poc[0:64, hs], rbr[0:64, hs])
                            cast_f(nm8[:, 0, :], nm[0:64, :])
                            nc.sync.dma_start(o28[64:128, 0, hp, cs], nm8[:, 0, :])
                            sub_e.tensor_sub(nm8[:, 1, :], nm[0:64, :], nm8[:, 0, :])
                            nc.sync.dma_start(o28[64:128, 1, hp, cs], nm8[:, 1, :])
                if debug and c == 0 and hp == 0:
                    nc.sync.dma_start(dbg["poc0"], poc[:])
                    nc.sync.dma_start(dbg["rbr0"], rbr[0:64, :])
                yield 100

        post = {}

        def open_post_pools():
            post["ost"] = ctx.enter_context(tc.tile_pool(name="ost", bufs=6))
            post["psP"] = ctx.enter_context(tc.tile_pool(name="psP", bufs=2, space="PSUM"))

        def gen_outproj_chunk(c, tail=False, pskey="psP", bs=range(4)):
            for b in bs:
                for n2 in range(2):  # pairs of 512-wide e-slices -> one DMA
                    st = post["ost"].tile(
                        [128, 1024], F16, tag="st", name=f"st_{c}_{b}_{n2}"
                    )
                    for nn in range(2):
                        n = 2 * n2 + nn
                        pp = post[pskey].tile(
                            [128, 512], FP, tag="pp", name=f"pp_{c}_{b}_{n}"
                        )
                        cs128 = ds(512 * c + 128 * b, 128)
                        ns = ds(512 * n, 512)
                        nc.tensor.matmul(
                            pp[:], o28[:, 0, 0:2, cs128], wo8[:, 1, 0:2, ns],
                            start=True, stop=False, perf_mode=DR,
                        )
                        nc.tensor.matmul(
                            pp[:], o28[:, 0:2, 0, cs128], wo8[:, 0:2, 0, ns],
                            start=False, stop=False, perf_mode=DR,
                        )
                        nc.tensor.matmul(
                            pp[:], o28[:, 0:2, 1, cs128], wo8[:, 0:2, 1, ns],
                            start=False, stop=True, perf_mode=DR,
                        )
                        # in the pure-PE tail alternate evacuation engines so
                        # the psum ring keeps pace with the matmuls
                        if tail and nn == 1:
                            nc.scalar.copy(st[:, ds(512, 512)], pp[:])
                        else:
                            nc.vector.tensor_copy(st[:, ds(512 * nn, 512)], pp[:])
                        yield 320
                    eng = (nc.sync, nc.gpsimd)[n2] if tail else nc.sync
                    eng.dma_start(
                        out[ds(128 * (4 * c + b), 128), ds(1024 * n2, 1024)], st[:]
                    )

        def chain(*gens):
            for g in gens:
                yield from g

        def closer():
            phaseP.close()
            open_post_pools()
            return
            yield  # pragma: no cover

        def weave(ga, gb, wa=1.0, wb=1.0):
            # proportional-progress interleave of two emission streams:
            # step the stream with the smaller fraction-complete so a short
            # filler spreads across the whole window instead of front-loading
            ta = tb = 0.0
            da = db = False
            while not (da and db):
                if db or (not da and ta / wa <= tb / wb):
                    try:
                        ta += next(ga)
                    except StopIteration:
                        da = True
                else:
                    try:
                        tb += next(gb)
                    except StopIteration:
                        db = True

        def run(g):
            for _ in g:
                pass

        # ---- pipeline: P0 [P1|A0] [P2|A1] [P3,close,O0|A2] [O1,O2|A3] O3 --
        run(gen_proj_quarter0())
        phase0.close()
        psA = phaseP.enter_context(tc.tile_pool(name="psA", bufs=2, space="PSUM"))
        psS = phaseA.enter_context(
            tc.tile_pool(name="psS", bufs=2, space="PSUM", side="right")
        )
        psO = phaseA.enter_context(
            tc.tile_pool(name="psO", bufs=1, space="PSUM", side="right")
        )
        weave(gen_proj_quarter(1), gen_attention_chunk(0))
        weave(gen_proj_quarter(2), gen_attention_chunk(1))
        weave(
            chain(gen_proj_quarter(3), closer(), gen_outproj_chunk(0)),
            gen_attention_chunk(2),
            wa=13.5,
            wb=16.6,
        )
        weave(
            chain(gen_outproj_chunk(1), gen_outproj_chunk(2, bs=range(3))),
            gen_attention_chunk(3),
            wa=8.9,
            wb=22.1,
        )
        # attention psum freed -> deep out-proj ring; the O2 remainder hides
        # the last normalize chain before O3 starts
        phaseA.close()
        post["psP2"] = ctx.enter_context(
            tc.tile_pool(name="psP2", bufs=4, space="PSUM", side="right")
        )
        run(gen_outproj_chunk(2, tail=True, pskey="psP2", bs=range(3, 4)))
        # PE-warming matmuls: chunk 3's out-projection can't start until its
        # normalize -> fp8 split -> upper-half DMA chain completes (~4us).
        # Discarded fp8 matmuls keep the array busy so the p-state ramp stays
        # hot and the tail runs at full clock the moment o28 is ready.
        N_WARM = 16
        if N_WARM:
            psW = ctx.enter_context(
                tc.tile_pool(name="psW", bufs=1, space="PSUM", side="right")
            )
            warm = psW.tile([128, 512], FP, tag="warm", name="warm")
            for _ in range(N_WARM):
                nc.tensor.matmul(
                    warm[:, 0:256], wo8[:, 0, 0, 0:128], wo8[:, 0, 0, 256:512],
                    start=True, stop=True,
                )
        run(gen_outproj_chunk(3, tail=True, pskey="psP2"))
        if debug:
            for name, tile in (
                ("q01s", q01s), ("q23s", q23s), ("ks", ks),
                ("v65", v_sb), ("o2a", o2a), ("o2b", o2b),
            ):
                nc.sync.dma_start(dbg[name], tile[:])

    nc.compile()
    return nc


def get_module(debug=False):
    key = ("nc", debug)
    if key not in _CACHE:
        _CACHE[key] = _build_module(debug=debug)
    return _CACHE[key]


def _pack16(x):
    # [16*128, N] -> [128, 16*N] with [p, N*g + n] = x[128*g + p, n]
    n = x.shape[1]
    return (
        np.ascontiguousarray(x.reshape(16, 128, n).transpose(1, 0, 2)).reshape(128, 16 * n)
    )


def _split8(x):
    # f32 -> (hi, lo) e4m3 with x ~= hi + lo
    hi = x.astype(E4)
    lo = (x - hi.astype(np.float32)).astype(E4)
    return hi, lo


def prep_inputs(hidden_states, freqs_cis, wqkv, wo):
    h = np.asarray(hidden_states, dtype=np.float32)[0]  # [S, D]
    fc = np.asarray(freqs_cis, dtype=np.float32)  # [S, 32, 2]
    wqkv = np.asarray(wqkv, dtype=np.float32)  # [3072, D]
    wo = np.asarray(wo, dtype=np.float32)  # [D, D]

    hT_f = _pack16(np.ascontiguousarray(h.T))  # [128, 16*S] f32
    hT_hi, hT_lo = _split8(hT_f)

    cos = fc[:, :, 0]  # [S, 32]
    sin = fc[:, :, 1]
    cos_ext = np.repeat(cos, 2, axis=1).T  # [64, S]
    sgn = np.where(np.arange(HD) % 2 == 0, -1.0, 1.0).astype(np.float32)[:, None]
    sin_ext = np.repeat(sin, 2, axis=1).T * sgn  # sin'[d, s]
    rqq_np = (
        np.concatenate([np.tile(cos_ext, (2, 1)), np.tile(sin_ext, (2, 1))], axis=1)
        / 32.0
    ).astype(np.float16)  # [128, 4096], 1/32 scale
    idf_np = np.zeros((128, 64), dtype=np.float32)
    idf_np[64:128] = np.eye(64, dtype=np.float32)
    tri_np = np.concatenate(
        [
            (np.arange(128)[:, None] <= np.arange(128)[None, :]).astype(np.float16),
            np.eye(128, dtype=np.float16),
        ],
        axis=1,
    )  # [128, 256]: triangle | identity

    in_maps = []
    for i in range(NCORES):
        wl = np.concatenate(
            [
                wqkv[256 * i : 256 * i + 256] * 32.0,
                wqkv[D + 64 * i : D + 64 * i + 64] * 32.0,
                wqkv[D + KV_SIZE + 64 * i : D + KV_SIZE + 64 * i + 64] * 64.0,
            ],
            axis=0,
        )  # [384, D], e4m3-friendly scales
        wq_f = _pack16(np.ascontiguousarray(wl.T))  # [128, 16*384] f32
        wq_hi, wq_lo = _split8(wq_f)
        woT = np.ascontiguousarray(wo[:, 256 * i : 256 * i + 256].T) * 64.0  # [256, D]
        wo_f = np.ascontiguousarray(woT.reshape(2, 128, D).transpose(1, 0, 2)).reshape(
            128, 2 * D
        )
        wo_hi, wo_lo = _split8(wo_f)
        in_maps.append(
            {
                "hT_hi": hT_hi,
                "hT_lo": hT_lo,
                "wq_hi": wq_hi,
                "wq_lo": wq_lo,
                "wo_hi": wo_hi,
                "wo_lo": wo_lo,
                "rqq": rqq_np,
                "tri": tri_np,
                "idf": idf_np,
            }
        )
    return in_maps


def run_on_hw(in_maps, trace=False, **kw):
    from concourse.bass_utils import run_bass_kernel_spmd

    nc = get_module()
    return run_bass_kernel_spmd(nc, in_maps, list(range(NCORES)), trace=trace, **kw)


def kernel(hidden_states, freqs_cis, wqkv, wo):
    in_maps = prep_inputs(hidden_states, freqs_cis, wqkv, wo)
    res = run_on_hw(in_maps)
    acc = np.zeros((S, D), dtype=np.float64)
    for r in res.results:
        acc += np.asarray(r["out"], dtype=np.float64)
    return (acc / OUT_DESCALE).astype(np.float32).reshape(1, S, D)


# revision 49
# speedup vs baseline: 1.0656x; 1.0656x over previous
"""Tensor-parallel (over GQA head groups) multi-head attention for 8 trn2 cores.

Each core owns 4 query heads + their shared kv head (one GQA group), the
matching 384 rows of wqkv and 256 columns of wo.  Every core computes a full
[S, D] partial of the output projection; the host sums the 8 partials.

v3: fp8e4 DoubleRow matmuls (0.5 PE cycles/row) for the qkv projection and
the output projection, with 3-term error compensation:

    A·B = A_hi·B_hi + (A_hi·B_lo + A_lo·B_hi)        (A_lo·B_lo dropped)

where X_hi = e4m3(X), X_lo = e4m3(X - X_hi).  The hi·hi term packs two
k-tiles per DoubleRow instruction; the cross term packs the two products of
ONE k-tile as the DoubleRow pair.  Net PE cost is 0.75x of one fp16 pass at
~0.2% relative error.  h/wqkv/wo are split on the host; the attention output
o2 is split on-device (ACT cast + DVE subtract).  Attention (scores, exp,
AV) stays fp16 exactly as v2.

Scale bookkeeping (e4m3 subnormal floor is 2^-6; weights must be scaled up):
  wq' rows = 32*wqkv_q | 32*wqkv_k | 64*wqkv_v   (no 1/sqrt(hd) in weights)
  rope tables carry 1/32  -> q01s/q23s/ks at true scale
  exp computed as exp(s/8 - 1): scale=1/8 is the scores 1/sqrt(hd),
    bias -1 keeps ex, the denominator and its f16 reciprocal in range
  v psum (64x) evacuated with activation scale 1/64 -> v at true scale
  v65 ones column = 1/16 -> normalized o2 tiles carry 16x (e4m3 range ~85)
  wo' = 64*wo; final psum is 1024x; the host divides the partials by 1024.
"""

import sys

if "/opt/trn_rl_repo" not in sys.path:
    sys.path.insert(0, "/opt/trn_rl_repo")

import ml_dtypes
import numpy as np

S = 2048
D = 2048
HD = 64
N_HEAD = 32
N_KV = 8
NCORES = 8
QH_PER_CORE = N_HEAD // NCORES  # 4
KV_SIZE = N_KV * HD  # 512
E4 = ml_dtypes.float8_e4m3
OUT_DESCALE = 1024.0  # 16 (o2 tiles) * 64 (wo)

_CACHE = {}


def _build_module(debug=False):
    from contextlib import ExitStack

    import concourse.mybir as mybir
    import concourse.tile as tile
    from concourse import bacc
    from concourse.bass import ds

    FP = mybir.dt.float32
    F16 = mybir.dt.float16
    F8 = mybir.dt.float8e4
    DR = mybir.MatmulPerfMode.DoubleRow
    EXP = mybir.ActivationFunctionType.Exp

    nc = bacc.Bacc(
        "TRN2",
        target_bir_lowering=False,
        debug=False,
        enable_asserts=False,
        num_devices=NCORES,
    )
    # register the exp bias constant (softmax shift, see emit_expav)
    _c = nc.alloc_sbuf_tensor("const-float32-neg1", [128, 1], FP)
    nc.gpsimd.memset(_c.ap(), -1.0)
    nc.const_aps.aps[(FP, -1.0)] = _c.ap()
    # 1/16 ones-column value for the softmax denominator accumulator
    _c16 = nc.alloc_sbuf_tensor("const-f16-inv16", [128, 1], F16)
    nc.gpsimd.memset(_c16.ap(), 0.0625)
    inv16 = _c16.ap()
    nc.all_engine_barrier()

    # [p, 2048*g + s] = e4m3(hidden)[s, 128*g + p], hi and lo halves
    hT_hi = nc.dram_tensor("hT_hi", [128, 16 * S], F8, kind="ExternalInput").ap()
    hT_lo = nc.dram_tensor("hT_lo", [128, 16 * S], F8, kind="ExternalInput").ap()
    # [p, 384*g + r] = wq'[r, 128*g + p]; r: 0-255 q, 256-319 k, 320-383 v
    wq_hi = nc.dram_tensor("wq_hi", [128, 16 * 384], F8, kind="ExternalInput").ap()
    wq_lo = nc.dram_tensor("wq_lo", [128, 16 * 384], F8, kind="ExternalInput").ap()
    # [p, 2048*u + e] = 64*wo[e, 256*core + 128*u + p]
    wo_hi = nc.dram_tensor("wo_hi", [128, 2 * 2048], F8, kind="ExternalInput").ap()
    wo_lo = nc.dram_tensor("wo_lo", [128, 2 * 2048], F8, kind="ExternalInput").ap()
    # rope tables, 1/32 scale: cols 0:2048 cos/32, 2048:4096 sin'/32
    rqq = nc.dram_tensor("rqq", [128, 2 * S], F16, kind="ExternalInput").ap()
    # cols 0:128 tri[p, f] = (p <= f); cols 128:256 identity[p, f] = (p == f)
    tri = nc.dram_tensor("tri", [128, 256], F16, kind="ExternalInput").ap()
    # f32 identity in rows 64-127 (rhs of the f32r v transposes)
    idf = nc.dram_tensor("idf", [128, 64], mybir.dt.float32r, kind="ExternalInput").ap()
    out = nc.dram_tensor("out", [S, D], F16, kind="ExternalOutput").ap()
    if debug:
        dbg = {
            name: nc.dram_tensor(f"dbg_{name}", shape, F16, kind="ExternalOutput").ap()
            for name, shape in (
                ("q01s", [128, S]),
                ("q23s", [128, S]),
                ("ks", [128, S]),
                ("v65", [128, 16 * 65]),
                ("o2a", [128, S]),
                ("o2b", [128, S]),
                ("ex0", [128, 8 * 1024]),
                ("sc0", [128, 8 * 1024]),
                ("po0", [65, 1024]),
                ("poc0", [65, 1024]),
                ("rbr0", [64, 1024]),
            )
        }

    with tile.TileContext(nc) as tc, ExitStack() as ctx:
        const = ctx.enter_context(tc.tile_pool(name="const", bufs=1))
        # [p, lo/hi, g%4, r]
        wqt = [
            const.tile([128, 2, 4, 384], F8, tag=f"wq{k}", name=f"wq_sb{k}")
            for k in range(4)
        ]

        def wq_hi_pair(g2, m):
            # hi weights for the k-tile pair (g2, g2+1); [128, 2, 128]
            k, lj = divmod(g2, 4)
            return wqt[k][:, 1, lj : lj + 2, 128 * m : 128 * m + 128]

        def wq_cross(g, m):
            # (lo, hi) weights of one k-tile; [128, 2, 128]
            k, lj = divmod(g, 4)
            return wqt[k][:, 0:2, lj, 128 * m : 128 * m + 128]

        F32R = mybir.dt.float32r
        # [p, lo/hi, u, e]
        wo8 = const.tile([128, 2, 2, 2048], F8, tag="wo")
        rqq_sb = const.tile([128, 4096], F16, tag="rqq")
        tri_sb = const.tile([128, 256], F16, tag="tri")
        idf_sb = const.tile([128, 64], FP, tag="idf")
        ident64 = idf_sb[64:128, :].bitcast(F32R)

        qsw = ctx.enter_context(tc.tile_pool(name="qsw", bufs=1))
        q01s = qsw.tile([128, S], F16, tag="q01s")
        q23s = qsw.tile([128, S], F16, tag="q23s")
        ks = qsw.tile([128, S], F16, tag="ks")
        v_sb = qsw.tile([128, 16 * 65], F16, tag="v")
        o2a = qsw.tile([128, S], F16, tag="o2a")
        o2b = qsw.tile([128, S], F16, tag="o2b")
        # [p, hi/lo, a/b, s] fp8 split of the (16x-scaled) attention output
        o28 = qsw.tile([128, 2, 2, S], F8, tag="o28")
        v65 = v_sb.rearrange("p (j c) -> p j c", c=65)

        # attention pools (live through the whole kernel)
        expool = ctx.enter_context(tc.tile_pool(name="ex", bufs=4))
        rspool = ctx.enter_context(tc.tile_pool(name="rs", bufs=2))
        rbpool = ctx.enter_context(tc.tile_pool(name="rb", bufs=2))
        nmpool = ctx.enter_context(tc.tile_pool(name="nm", bufs=2))
        # attention psum pools are created after quarter 0 releases its
        # 3-bank ring (right side of the arena; release is LIFO per side)
        phaseA = ExitStack()
        psS = psO = None

        # projection-phase pools (close before the out-projection opens)
        phaseP = ExitStack()
        hpool = phaseP.enter_context(tc.tile_pool(name="hp", bufs=24))
        qraw = phaseP.enter_context(tc.tile_pool(name="qraw", bufs=1))
        q01 = qraw.tile([128, S], F16, tag="q01")
        q23 = qraw.tile([128, S], F16, tag="q23")
        kv = qraw.tile([128, S], F16, tag="kv")  # rows 0:64 = k (rope input)
        vraw = qraw.tile([128, S], FP, tag="vraw")  # rows 64:128 = v, f32
        scpool = phaseP.enter_context(tc.tile_pool(name="sc", bufs=4))
        phase0 = ExitStack()
        psA0 = phase0.enter_context(tc.tile_pool(name="psA0", bufs=1, space="PSUM"))
        psA = None

        SWAP_MASK = [i ^ 1 for i in range(32)]

        def rope_quarter(dst, raw, p, costab, sintab, q, nm):
            # dst = raw * cos + pairswap(raw) * sin' on [0:p, 512q:512q+512]
            cs = ds(512 * q, 512)
            sw = scpool.tile([128, 512], F16, tag="sc", name=f"sw_{nm}{q}")
            nc.vector.stream_shuffle(sw[0:p, :], raw[0:p, cs], SWAP_MASK)
            t0 = scpool.tile([128, 512], F16, tag="sc", name=f"t0_{nm}{q}")
            nc.vector.tensor_mul(t0[0:p, :], raw[0:p, cs], costab[0:p, cs])
            nc.vector.tensor_mul(sw[0:p, :], sw[0:p, :], sintab[0:p, cs])
            nc.vector.tensor_add(dst[0:p, cs], t0[0:p, :], sw[0:p, :])

        # global DMA plan: one ordered stream of large transfers.  hT comes as
        # 2048-column quads ([128, 4, 512] strided, >=512B contiguous runs),
        # hi half before lo half so the hi*hi matmuls can start early.
        hT_hi_r = hT_hi.rearrange("p (g s) -> p g s", s=2048)
        hT_lo_r = hT_lo.rearrange("p (g s) -> p g s", s=2048)
        rq_r = rqq.rearrange("p (h s) -> p h s", s=2048)
        hq_tiles = {}

        def dma_hq(q, t, which, half=None, eng=None):
            # quad t of quarter q, which: 0 hi / 1 lo
            if half is None:
                gs, n = 4 * t, 4
            else:
                gs, n = 4 * t + 2 * half, 2
            tile = hq_tiles.get((q, t))
            if tile is None:
                tile = hpool.tile([128, 2, 4, 512], F8, tag="hc", name=f"hq_{q}_{t}")
                hq_tiles[(q, t)] = tile
            src_r = hT_hi_r if which == 0 else hT_lo_r
            (eng or nc.sync).dma_start(
                tile[:, which, gs - 4 * t : gs - 4 * t + n, :],
                src_r[:, gs : gs + n, ds(512 * q, 512)],
            )

        def hq_hi_pair(q, g2):
            k, lj = divmod(g2, 4)
            return hq_tiles[(q, k)][:, 0, lj : lj + 2, :]

        def hq_cross(q, g):
            k, lj = divmod(g, 4)
            return hq_tiles[(q, k)][:, 0:2, lj, :]

        def dma_wq(k, which, eng=None):
            # which: 0 lo / 1 hi
            src = (wq_lo if which == 0 else wq_hi)[:, ds(1536 * k, 1536)]
            (eng or nc.sync).dma_start(
                wqt[k][:, which], src.rearrange("p (g r) -> p g r", r=384)
            )

        rqsb_r = rqq_sb.rearrange("p (h s) -> p h s", s=2048)

        def dma_rq(q):
            nc.sync.dma_start(
                rqsb_r[:, :, ds(512 * q, 512)], rq_r[:, :, ds(512 * q, 512)]
            )

        def dma_wo(which):
            src = (wo_lo if which == 0 else wo_hi)
            nc.sync.dma_start(
                wo8[:, which], src.rearrange("p (u e) -> p u e", e=2048)
            )

        def dma_wq0h(half):
            # startup split: pairs (0,1) land before (2,3)
            src = wq_hi[:, ds(768 * half, 768)]
            nc.sync.dma_start(
                wqt[0][:, 1, 2 * half : 2 * half + 2, :],
                src.rearrange("p (g r) -> p g r", r=384),
            )

        # quarter-0 feed is split across the SP and ACT DGE queues (same-
        # queue transfers serialize in the DMA model; ACT is idle until the
        # first evacuation): hq-hi + wq-lo on SP, wq-hi + hq-lo on ACT, so
        # each quad's supply is ~1.3us/queue vs 1.9us of PE work.
        dma_plan = [
            ("wq0ha", lambda: dma_wq0h(0)),
            ("hq00ha", lambda: dma_hq(0, 0, 0, 0)),
            ("wq0hb", lambda: dma_wq0h(1)),
            ("hq00hb", lambda: dma_hq(0, 0, 0, 1)),
            ("hq00l", lambda: dma_hq(0, 0, 1, eng=nc.gpsimd)),
            ("wq0l", lambda: dma_wq(0, 0)),
            ("wq1h", lambda: dma_wq(1, 1, eng=nc.gpsimd)),
            ("hq01h", lambda: dma_hq(0, 1, 0)),
            ("hq01l", lambda: dma_hq(0, 1, 1, eng=nc.gpsimd)),
            ("wq1l", lambda: dma_wq(1, 0)),
            ("wq2h", lambda: dma_wq(2, 1, eng=nc.gpsimd)),
            ("hq02h", lambda: dma_hq(0, 2, 0)),
            ("hq02l", lambda: dma_hq(0, 2, 1, eng=nc.gpsimd)),
            ("wq2l", lambda: dma_wq(2, 0)),
            ("wq3h", lambda: dma_wq(3, 1, eng=nc.gpsimd)),
            ("hq03h", lambda: dma_hq(0, 3, 0)),
            ("hq03l", lambda: dma_hq(0, 3, 1, eng=nc.gpsimd)),
            ("wq3l", lambda: dma_wq(3, 0)),
            ("tri", lambda: (nc.sync.dma_start(tri_sb[:], tri),
                             nc.sync.dma_start(idf_sb[:].bitcast(F32R), idf))),
            ("rq0", lambda: dma_rq(0)),
            ("hq10", lambda: (dma_hq(1, 0, 0), dma_hq(1, 0, 1))),
            ("hq11", lambda: (dma_hq(1, 1, 0), dma_hq(1, 1, 1))),
            ("hq12", lambda: (dma_hq(1, 2, 0), dma_hq(1, 2, 1))),
            ("rq1", lambda: dma_rq(1)),
            ("hq13", lambda: (dma_hq(1, 3, 0), dma_hq(1, 3, 1))),
            ("hq20", lambda: (dma_hq(2, 0, 0), dma_hq(2, 0, 1))),
            ("wo0", lambda: dma_wo(1)),
            ("hq21", lambda: (dma_hq(2, 1, 0), dma_hq(2, 1, 1))),
            ("hq22", lambda: (dma_hq(2, 2, 0), dma_hq(2, 2, 1))),
            ("wo1", lambda: dma_wo(0)),
            ("rq2", lambda: dma_rq(2)),
            ("hq23", lambda: (dma_hq(2, 3, 0), dma_hq(2, 3, 1))),
            ("hq30", lambda: (dma_hq(3, 0, 0), dma_hq(3, 0, 1))),
            ("hq31", lambda: (dma_hq(3, 1, 0), dma_hq(3, 1, 1))),
            ("hq32", lambda: (dma_hq(3, 2, 0), dma_hq(3, 2, 1))),
            ("rq3", lambda: dma_rq(3)),
            ("hq33", lambda: (dma_hq(3, 3, 0), dma_hq(3, 3, 1))),
        ]
        plan_pos = {key: idx for idx, (key, _) in enumerate(dma_plan)}
        feed_state = {"next": 0}

        def feed_until(key):
            stop = plan_pos[key] + 1
            while feed_state["next"] < stop:
                dma_plan[feed_state["next"]][1]()
                feed_state["next"] += 1

        def proj_quad(q, t, ms, pts, started, last=()):
            # all fp8 matmuls of quad t for the m-passes in ms.
            # last: set of m whose final matmul gets stop=True.
            # jj/lg outer so the DMA pieces are consumed in arrival order.
            for jj in range(2):
                g2 = 4 * t + 2 * jj
                for m in ms:
                    nc.tensor.matmul(
                        pts[m][:], wq_hi_pair(g2, m), hq_hi_pair(q, g2),
                        start=(m not in started), stop=False, perf_mode=DR,
                    )
                    started.add(m)
            for lg in range(4):
                g = 4 * t + lg
                for m in ms:
                    nc.tensor.matmul(
                        pts[m][:], wq_cross(g, m), hq_cross(q, g),
                        start=False, stop=(m in last and lg == 3), perf_mode=DR,
                    )

        def emit_vtrans(q, vtp):
            # v transposes for quarter q: f32r through the projection psum
            # ring slot, then one f16 evacuation into v65
            vt = vtp.tile([128, 512], FP, tag=vtp_tag(vtp), name=f"vt_{q}")
            for jj in range(4):
                j = 4 * q + jj
                nc.tensor.transpose(
                    vt[:, ds(64 * jj, 64)].bitcast(F32R),
                    vraw[64:128, ds(128 * j, 128)].bitcast(F32R),
                    ident64,
                )
            nc.vector.tensor_copy(
                v65[:, 4 * q : 4 * q + 4, 0:64],
                vt[:, 0:256].rearrange("p (jj c) -> p jj c", c=64),
            )
            nc.vector.tensor_copy(
                v65[:, 4 * q : 4 * q + 4, 64:65],
                inv16[:, 0:1][:, None, :].to_broadcast([128, 4, 1]),
            )

        def vtp_tag(pool):
            return "p0" if pool is psA0 else "pj"

        def evac_kv(pts_m, q):
            cs = ds(512 * q, 512)
            nc.scalar.copy(kv[0:64, cs], pts_m[0:64, :])
            # v psum is 64x; descale to true scale on the way out
            nc.scalar.mul(vraw[64:128, cs].bitcast(F32R), pts_m[64:128, :], 1.0 / 64.0)

        def gen_proj_quarter0():
            # quarter 0 runs all three m-passes interleaved at the pace the
            # DMA stream can sustain (the front is inherently supply-bound)
            pts = [
                psA0.tile([128, 512], FP, tag=f"p{m}", name=f"pj0_{m}")
                for m in range(3)
            ]
            started = set()
            feed_until("hq00hb")
            for t in range(3):
                # hi*hi needs this quad's hi pieces; cross needs lo too
                feed_until(["wq0l", "wq1l", "wq2l"][t])
                proj_quad(0, t, (0, 1, 2), pts, started)
                feed_until(["wq3l", "tri", "rq0"][t])
                yield 1280
            # quad 3 m-serially, kv first: rope-k (which gates the first
            # attention scores) starts earliest
            cos_t, sin_t = rqq_sb[:, 0:2048], rqq_sb[:, 2048:4096]
            feed_until("wq3l")
            for m, dst, dsts, p, nm in (
                (2, kv, ks, 64, "k"),
                (0, q01, q01s, 128, "q01"),
                (1, q23, q23s, 128, "q23"),
            ):
                proj_quad(0, 3, (m,), pts, started, last={m})
                if m == 2:
                    evac_kv(pts[m], 0)
                else:
                    nc.scalar.copy(dst[:, 0:512], pts[m][:])
                rope_quarter(dsts, dst, p, cos_t, sin_t, 0, nm)
                if m == 2:
                    nc.sync.dma_start(ks[64:128, 0:512], ks[0:64, 0:512])
                feed_until("hq10")
                yield 940
            emit_vtrans(0, psA0)
            yield 80
            feed_until("hq11")
            yield 120

        def gen_proj_quarter(q):
            feed_until(f"hq{q}1")
            pts = {}
            for m in (0, 1):
                pts[m] = psA.tile([128, 512], FP, tag="pj", name=f"pj_{q}_{m}")
            started = set()
            for t in range(4):
                if t >= 2:
                    feed_until(f"hq{q}{min(t + 1, 3)}")
                proj_quad(q, t, (0, 1), pts, started,
                          last=({0, 1} if t == 3 else ()))
                yield 1280
            cs = ds(512 * q, 512)
            nc.scalar.copy(q01[:, cs], pts[0][:])
            rope_quarter(q01s, q01, 128, rqq_sb[:, 0:2048], rqq_sb[:, 2048:4096], q, "q01")
            yield 300
            nc.scalar.copy(q23[:, cs], pts[1][:])
            rope_quarter(q23s, q23, 128, rqq_sb[:, 0:2048], rqq_sb[:, 2048:4096], q, "q23")
            yield 300
            pt2 = psA.tile([128, 512], FP, tag="pj", name=f"pj_{q}_2")
            started2 = set()
            for t in range(4):
                proj_quad(q, t, (2,), pts={2: pt2}, started=started2,
                          last=({2} if t == 3 else ()))
                # deep-prefetch the next quarter while the psum ring is
                # the only DMA consumer
                if q < 3:
                    feed_until(f"hq{q + 1}{min(t, 3)}")
                yield 640
            evac_kv(pt2, q)
            rope_quarter(ks, kv, 64, rqq_sb[:, 0:2048], rqq_sb[:, 2048:4096], q, "k")
            yield 300
            # duplicate rotated k at partitions 64-127 (odd heads' score
            # matmuls read lhsT/rhs both at base 64)
            nc.sync.dma_start(ks[64:128, cs], ks[0:64, cs])
            emit_vtrans(q, psA)
            if q < 3:
                feed_until(f"hq{q + 1}1")
            yield 120

        def gen_attention_chunk(c):
            nj = 4 * c + 4
            for hp in range(2):
                po = psO.tile([65, 1024], FP, tag="po", name=f"po_{c}_{hp}")

                def emit_scores(j):
                    r = j - 4 * c  # >= 0 on diagonal blocks
                    off = 128 * r if r >= 0 else 0
                    ps = psS.tile([128, 1024], FP, tag="ps", name=f"ps_{c}_{hp}_{j}")
                    for hh in range(2):
                        h = 2 * hp + hh
                        qt = q01s if h < 2 else q23s
                        base = 64 * (h % 2)
                        nc.tensor.matmul(
                            ps[:, ds(512 * hh + off, 512 - off)],
                            ks[base : base + 64, ds(128 * j, 128)],
                            qt[base : base + 64, ds(512 * c + off, 512 - off)],
                        )
                    return ps, off, r >= 0

                def emit_expav(j, ps, off, diag):
                    # exp(s/8 - 2): 1/8 is the scores 1/sqrt(hd); softmax is
                    # shift-invariant and the bias keeps the f16 sums and the
                    # denominator reciprocal in range
                    ex = expool.tile([128, 1024], F16, tag="ex", name=f"ex_{c}_{hp}_{j}")
                    if not diag:
                        nc.scalar.activation(ex[:], ps[:], EXP, bias=-1.0, scale=0.125)
                    else:
                        w = 512 - off
                        psv = ps.rearrange("p (h w) -> p h w", w=512)[:, :, ds(off, w)]
                        exv = ex.rearrange("p (h w) -> p h w", w=512)[:, :, ds(off, w)]
                        nc.scalar.activation(exv, psv, EXP, bias=-1.0, scale=0.125)
                        exd = ex.rearrange("p (h w) -> p h w", w=512)[:, :, ds(off, 128)]
                        nc.vector.tensor_mul(
                            exd,
                            exd,
                            tri_sb[:, 0:128][:, None, :].to_broadcast([128, 2, 128]),
                        )
                    if debug and c == 0:
                        sl = ds(1024 * (4 * hp + j), 1024)
                        nc.sync.dma_start(dbg["ex0"][:, sl], ex[:])
                        sc16 = expool.tile(
                            [128, 1024], F16, tag="ex", name=f"scd_{hp}_{j}"
                        )
                        nc.vector.tensor_copy(sc16[:], ps[:])
                        nc.sync.dma_start(dbg["sc0"][:, sl], sc16[:])
                    for hh in range(2):
                        nc.tensor.matmul(
                            po[0:65, ds(512 * hh + off, 512 - off)],
                            v_sb[:, ds(65 * j, 65)],
                            ex[:, ds(512 * hh + off, 512 - off)],
                            start=(j == 0),
                            stop=(j == nj - 1),
                            skip_group_check=True,
                        )

                # one-j lookahead: scores(j+1) land on the PE between
                # scores(j) and av(j) so the exp never stalls the PE
                def jcost(j):
                    # PE ns of one scores OR av pair at this block's trim
                    r = j - 4 * c
                    off = 128 * r if r >= 0 else 0
                    return int((512 - off) * 0.833)

                prev = emit_scores(0)
                for j in range(1, nj):
                    cur = emit_scores(j)
                    emit_expav(j - 1, *prev)
                    prev = cur
                    yield jcost(j) + jcost(j - 1)
                emit_expav(nj - 1, *prev)
                yield jcost(nj - 1)
                # evacuate the accumulator so the bank frees for the next
                # head pair, then normalize: reciprocal of the sums row,
                # broadcast back into the evacuated po bank, then per half:
                # f16 o2 piece -> fp8 hi (ACT cast) + fp8 lo (DVE subtract)
                poc = rspool.tile([65, 1024], F16, tag="rs", name=f"poc_{c}_{hp}")
                if debug and c == 0 and hp == 0:
                    pod = rspool.tile([65, 1024], F16, tag="pod", name="pod")
                    nc.vector.tensor_copy(pod[:], po[:])
                    nc.sync.dma_start(dbg["po0"], pod[:])
                nc.scalar.copy(poc[:, 512:1024], po[:, 512:1024])
                nc.vector.tensor_copy(poc[:, 0:512], po[:, 0:512])
                rbr = rbpool.tile([64, 1024], F16, tag="rbr", name=f"rbr_{c}_{hp}")
                dsttile = o2a if hp == 0 else o2b
                nm = nmpool.tile([64, 512], F16, tag="nm", name=f"nm_{c}_{hp}")
                nm8 = nmpool.tile([64, 2, 512], F8, tag="nm8", name=f"nm8_{c}_{hp}")
                cs = ds(512 * c, 512)
                for half in (1, 0):
                    hs = ds(512 * half, 512)
                    nc.tensor.matmul(
                        po[0:64, hs], tri_sb[64:65, 64:128], poc[64:65, hs],
                        start=True, stop=True,
                    )
                    # the fp8 hi/lo split runs on the (otherwise idle) Pool
                    # engine, except the last chunk where ACT/DVE are free
                    # and Pool's ~1us/op latency would stretch the tail; the
                    # upper-half DMAs dispatch as soon as each piece exists
                    if c == 3:
                        cast_f = nc.scalar.copy
                        sub_e = nc.gpsimd if half == 1 else nc.vector
                    else:
                        cast_f, sub_e = nc.gpsimd.tensor_copy, nc.gpsimd
                    with nc.allow_low_precision(reason="softmax denom recip f16"):
                        nc.vector.reciprocal(rbr[0:64, hs], po[0:64, hs])
                        if half == 0:
                            piece = dsttile[0:64, cs]
                            nc.vector.tensor_mul(piece, poc[0:64, hs], rbr[0:64, hs])
                            hi8 = o28[0:64, 0, hp, cs]
                            cast_f(hi8, piece)
                            sub_e.tensor_sub(o28[0:64, 1, hp, cs], piece, hi8)
                        else:
                            nc.vector.tensor_mul(nm[0:64, :], poc[0:64, hs], rbr[0:64, hs])
                            cast_f(nm8[:, 0, :], nm[0:64, :])
                            nc.sync.dma_start(o28[64:128, 0, hp, cs], nm8[:, 0, :])
                            sub_e.tensor_sub(nm8[:, 1, :], nm[0:64, :], nm8[:, 0, :])
                            nc.sync.dma_start(o28[64:128, 1, hp, cs], nm8[:, 1, :])
                if debug and c == 0 and hp == 0:
                    nc.sync.dma_start(dbg["poc0"], poc[:])
                    nc.sync.dma_start(dbg["rbr0"], rbr[0:64, :])
                yield 100

        post = {}

        def open_post_pools():
            post["ost"] = ctx.enter_context(tc.tile_pool(name="ost", bufs=6))
            post["psP"] = ctx.enter_context(tc.tile_pool(name="psP", bufs=2, space="PSUM"))

        def gen_outproj_chunk(c, tail=False, pskey="psP", bs=range(4)):
            for b in bs:
                for n2 in range(2):  # pairs of 512-wide e-slices -> one DMA
                    st = post["ost"].tile(
                        [128, 1024], F16, tag="st", name=f"st_{c}_{b}_{n2}"
                    )
                    for nn in range(2):
                        n = 2 * n2 + nn
                        pp = post[pskey].tile(
                            [128, 512], FP, tag="pp", name=f"pp_{c}_{b}_{n}"
                        )
                        cs128 = ds(512 * c + 128 * b, 128)
                        ns = ds(512 * n, 512)
                        nc.tensor.matmul(
                            pp[:], o28[:, 0, 0:2, cs128], wo8[:, 1, 0:2, ns],
                            start=True, stop=False, perf_mode=DR,
                        )
                        nc.tensor.matmul(
                            pp[:], o28[:, 0:2, 0, cs128], wo8[:, 0:2, 0, ns],
                            start=False, stop=False, perf_mode=DR,
                        )
                        nc.tensor.matmul(
                            pp[:], o28[:, 0:2, 1, cs128], wo8[:, 0:2, 1, ns],
                            start=False, stop=True, perf_mode=DR,
                        )
                        # in the pure-PE tail alternate evacuation engines so
                        # the psum ring keeps pace with the matmuls
                        if tail and nn == 1:
                            nc.scalar.copy(st[:, ds(512, 512)], pp[:])
                        else:
                            nc.vector.tensor_copy(st[:, ds(512 * nn, 512)], pp[:])
                        yield 320
                    eng = (nc.sync, nc.gpsimd)[n2] if tail else nc.sync
                    eng.dma_start(
                        out[ds(128 * (4 * c + b), 128), ds(1024 * n2, 1024)], st[:]
                    )

        def chain(*gens):
            for g in gens:
                yield from g

        def closer():
            phaseP.close()
            open_post_pools()
            return
            yield  # pragma: no cover

        def weave(ga, gb, wa=1.0, wb=1.0):
            # proportional-progress interleave of two emission streams:
            # step the stream with the smaller fraction-complete so a short
            # filler spreads across the whole window instead of front-loading
            ta = tb = 0.0
            da = db = False
            while not (da and db):
                if db or (not da and ta / wa <= tb / wb):
                    try:
                        ta += next(ga)
                    except StopIteration:
                        da = True
                else:
                    try:
                        tb += next(gb)
                    except StopIteration:
                        db = True

        def run(g):
            for _ in g:
                pass

        # ---- pipeline: P0 [P1|A0] [P2|A1] [P3,close,O0|A2] [O1,O2|A3] O3 --
        run(gen_proj_quarter0())
        phase0.close()
        psA = phaseP.enter_context(tc.tile_pool(name="psA", bufs=2, space="PSUM"))
        psS = phaseA.enter_context(
            tc.tile_pool(name="psS", bufs=2, space="PSUM", side="right")
        )
        psO = phaseA.enter_context(
            tc.tile_pool(name="psO", bufs=1, space="PSUM", side="right")
        )
        weave(gen_proj_quarter(1), gen_attention_chunk(0))
        weave(gen_proj_quarter(2), gen_attention_chunk(1))
        weave(
            chain(gen_proj_quarter(3), closer(), gen_outproj_chunk(0)),
            gen_attention_chunk(2),
            wa=13.5,
            wb=16.6,
        )
        weave(
            chain(gen_outproj_chunk(1), gen_outproj_chunk(2, bs=range(3))),
            gen_attention_chunk(3),
            wa=8.9,
            wb=22.1,
        )
        # attention psum freed -> deep out-proj ring; the O2 remainder hides
        # the last normalize chain before O3 starts
        phaseA.close()
        post["psP2"] = ctx.enter_context(
            tc.tile_pool(name="psP2", bufs=4, space="PSUM", side="right")
        )
        run(gen_outproj_chunk(2, tail=True, pskey="psP2", bs=range(3, 4)))
        # PE-warming matmuls: chunk 3's out-projection can't start until its
        # normalize -> fp8 split -> upper-half DMA chain completes (~4us).
        # Discarded fp8 matmuls keep the array busy so the p-state ramp stays
        # hot and the tail runs at full clock the moment o28 is ready.
        N_WARM = 16
        if N_WARM:
            psW = ctx.enter_context(
                tc.tile_pool(name="psW", bufs=1, space="PSUM", side="right")
            )
            warm = psW.tile([128, 512], FP, tag="warm", name="warm")
            for _ in range(N_WARM):
                nc.tensor.matmul(
                    warm[:, 0:256], wo8[:, 0, 0, 0:128], wo8[:, 0, 0, 256:512],
                    start=True, stop=True,
                )
        run(gen_outproj_chunk(3, tail=True, pskey="psP2"))
        if debug:
            for name, tile in (
                ("q01s", q01s), ("q23s", q23s), ("ks", ks),
                ("v65", v_sb), ("o2a", o2a), ("o2b", o2b),
            ):
                nc.sync.dma_start(dbg[name], tile[:])

    nc.compile()
    return nc


def get_module(debug=False):
    key = ("nc", debug)
    if key not in _CACHE:
        _CACHE[key] = _build_module(debug=debug)
    return _CACHE[key]


def _pack16(x):
    # [16*128, N] -> [128, 16*N] with [p, N*g + n] = x[128*g + p, n]
    n = x.shape[1]
    return (
        np.ascontiguousarray(x.reshape(16, 128, n).transpose(1, 0, 2)).reshape(128, 16 * n)
    )


def _split8(x):
    # f32 -> (hi, lo) e4m3 with x ~= hi + lo
    hi = x.astype(E4)
    lo = (x - hi.astype(np.float32)).astype(E4)
    return hi, lo


def prep_inputs(hidden_states, freqs_cis, wqkv, wo):
    h = np.asarray(hidden_states, dtype=np.float32)[0]  # [S, D]
    fc = np.asarray(freqs_cis, dtype=np.float32)  # [S, 32, 2]
    wqkv = np.asarray(wqkv, dtype=np.float32)  # [3072, D]
    wo = np.asarray(wo, dtype=np.float32)  # [D, D]

    hT_f = _pack16(np.ascontiguousarray(h.T))  # [128, 16*S] f32
    hT_hi, hT_lo = _split8(hT_f)

    cos = fc[:, :, 0]  # [S, 32]
    sin = fc[:, :, 1]
    cos_ext = np.repeat(cos, 2, axis=1).T  # [64, S]
    sgn = np.where(np.arange(HD) % 2 == 0, -1.0, 1.0).astype(np.float32)[:, None]
    sin_ext = np.repeat(sin, 2, axis=1).T * sgn  # sin'[d, s]
    rqq_np = (
        np.concatenate([np.tile(cos_ext, (2, 1)), np.tile(sin_ext, (2, 1))], axis=1)
        / 32.0
    ).astype(np.float16)  # [128, 4096], 1/32 scale
    idf_np = np.zeros((128, 64), dtype=np.float32)
    idf_np[64:128] = np.eye(64, dtype=np.float32)
    tri_np = np.concatenate(
        [
            (np.arange(128)[:, None] <= np.arange(128)[None, :]).astype(np.float16),
            np.eye(128, dtype=np.float16),
        ],
        axis=1,
    )  # [128, 256]: triangle | identity

    in_maps = []
    for i in range(NCORES):
        wl = np.concatenate(
            [
                wqkv[256 * i : 256 * i + 256] * 32.0,
                wqkv[D + 64 * i : D + 64 * i + 64] * 32.0,
                wqkv[D + KV_SIZE + 64 * i : D + KV_SIZE + 64 * i + 64] * 64.0,
            ],
            axis=0,
        )  # [384, D], e4m3-friendly scales
        wq_f = _pack16(np.ascontiguousarray(wl.T))  # [128, 16*384] f32
        wq_hi, wq_lo = _split8(wq_f)
        woT = np.ascontiguousarray(wo[:, 256 * i : 256 * i + 256].T) * 64.0  # [256, D]
        wo_f = np.ascontiguousarray(woT.reshape(2, 128, D).transpose(1, 0, 2)).reshape(
            128, 2 * D
        )
        wo_hi, wo_lo = _split8(wo_f)
        in_maps.append(
            {
                "hT_hi": hT_hi,
                "hT_lo": hT_lo,
                "wq_hi": wq_hi,
                "wq_lo": wq_lo,
                "wo_hi": wo_hi,
                "wo_lo": wo_lo,
                "rqq": rqq_np,
                "tri": tri_np,
                "idf": idf_np,
            }
        )
    return in_maps


def run_on_hw(in_maps, trace=False, **kw):
    from concourse.bass_utils import run_bass_kernel_spmd

    nc = get_module()
    return run_bass_kernel_spmd(nc, in_maps, list(range(NCORES)), trace=trace, **kw)


def kernel(hidden_states, freqs_cis, wqkv, wo):
    in_maps = prep_inputs(hidden_states, freqs_cis, wqkv, wo)
    res = run_on_hw(in_maps)
    acc = np.zeros((S, D), dtype=np.float64)
    for r in res.results:
        acc += np.asarray(r["out"], dtype=np.float64)
    return (acc / OUT_DESCALE).astype(np.float32).reshape(1, S, D)


# revision 50
# speedup vs baseline: 1.0701x; 1.0042x over previous
"""Tensor-parallel (over GQA head groups) multi-head attention for 8 trn2 cores.

Each core owns 4 query heads + their shared kv head (one GQA group), the
matching 384 rows of wqkv and 256 columns of wo.  Every core computes a full
[S, D] partial of the output projection; the host sums the 8 partials.

v3: fp8e4 DoubleRow matmuls (0.5 PE cycles/row) for the qkv projection and
the output projection, with 3-term error compensation:

    A·B = A_hi·B_hi + (A_hi·B_lo + A_lo·B_hi)        (A_lo·B_lo dropped)

where X_hi = e4m3(X), X_lo = e4m3(X - X_hi).  The hi·hi term packs two
k-tiles per DoubleRow instruction; the cross term packs the two products of
ONE k-tile as the DoubleRow pair.  Net PE cost is 0.75x of one fp16 pass at
~0.2% relative error.  h/wqkv/wo are split on the host; the attention output
o2 is split on-device (ACT cast + DVE subtract).  Attention (scores, exp,
AV) stays fp16 exactly as v2.

Scale bookkeeping (e4m3 subnormal floor is 2^-6; weights must be scaled up):
  wq' rows = 32*wqkv_q | 32*wqkv_k | 64*wqkv_v   (no 1/sqrt(hd) in weights)
  rope tables carry 1/32  -> q01s/q23s/ks at true scale
  exp computed as exp(s/8 - 1): scale=1/8 is the scores 1/sqrt(hd),
    bias -1 keeps ex, the denominator and its f16 reciprocal in range
  v psum (64x) evacuated with activation scale 1/64 -> v at true scale
  v65 ones column = 1/16 -> normalized o2 tiles carry 16x (e4m3 range ~85)
  wo' = 64*wo; final psum is 1024x; the host divides the partials by 1024.
"""

import sys

if "/opt/trn_rl_repo" not in sys.path:
    sys.path.insert(0, "/opt/trn_rl_repo")

import ml_dtypes
import numpy as np

S = 2048
D = 2048
HD = 64
N_HEAD = 32
N_KV = 8
NCORES = 8
QH_PER_CORE = N_HEAD // NCORES  # 4
KV_SIZE = N_KV * HD  # 512
E4 = ml_dtypes.float8_e4m3
OUT_DESCALE = 1024.0  # 16 (o2 tiles) * 64 (wo)

_CACHE = {}


def _build_module(debug=False):
    from contextlib import ExitStack

    import concourse.mybir as mybir
    import concourse.tile as tile
    from concourse import bacc
    from concourse.bass import ds

    FP = mybir.dt.float32
    F16 = mybir.dt.float16
    F8 = mybir.dt.float8e4
    DR = mybir.MatmulPerfMode.DoubleRow
    EXP = mybir.ActivationFunctionType.Exp

    nc = bacc.Bacc(
        "TRN2",
        target_bir_lowering=False,
        debug=False,
        enable_asserts=False,
        num_devices=NCORES,
    )
    # register the exp bias constant (softmax shift, see emit_expav)
    _c = nc.alloc_sbuf_tensor("const-float32-neg1", [128, 1], FP)
    nc.gpsimd.memset(_c.ap(), -1.0)
    nc.const_aps.aps[(FP, -1.0)] = _c.ap()
    # 1/16 ones-column value for the softmax denominator accumulator
    _c16 = nc.alloc_sbuf_tensor("const-f16-inv16", [128, 1], F16)
    nc.gpsimd.memset(_c16.ap(), 0.0625)
    inv16 = _c16.ap()
    nc.all_engine_barrier()

    # [p, 2048*g + s] = e4m3(hidden)[s, 128*g + p], hi and lo halves
    hT_hi = nc.dram_tensor("hT_hi", [128, 16 * S], F8, kind="ExternalInput").ap()
    hT_lo = nc.dram_tensor("hT_lo", [128, 16 * S], F8, kind="ExternalInput").ap()
    # [p, 384*g + r] = wq'[r, 128*g + p]; r: 0-255 q, 256-319 k, 320-383 v
    wq_hi = nc.dram_tensor("wq_hi", [128, 16 * 384], F8, kind="ExternalInput").ap()
    wq_lo = nc.dram_tensor("wq_lo", [128, 16 * 384], F8, kind="ExternalInput").ap()
    # [p, 2048*u + e] = 64*wo[e, 256*core + 128*u + p]
    wo_hi = nc.dram_tensor("wo_hi", [128, 2 * 2048], F8, kind="ExternalInput").ap()
    wo_lo = nc.dram_tensor("wo_lo", [128, 2 * 2048], F8, kind="ExternalInput").ap()
    # rope tables, 1/32 scale: cols 0:2048 cos/32, 2048:4096 sin'/32
    rqq = nc.dram_tensor("rqq", [128, 2 * S], F16, kind="ExternalInput").ap()
    # cols 0:128 tri[p, f] = (p <= f); cols 128:256 identity[p, f] = (p == f)
    tri = nc.dram_tensor("tri", [128, 256], F16, kind="ExternalInput").ap()
    # f32 identity in rows 64-127 (rhs of the f32r v transposes)
    idf = nc.dram_tensor("idf", [128, 64], mybir.dt.float32r, kind="ExternalInput").ap()
    out = nc.dram_tensor("out", [S, D], F16, kind="ExternalOutput").ap()
    if debug:
        dbg = {
            name: nc.dram_tensor(f"dbg_{name}", shape, F16, kind="ExternalOutput").ap()
            for name, shape in (
                ("q01s", [128, S]),
                ("q23s", [128, S]),
                ("ks", [128, S]),
                ("v65", [128, 16 * 65]),
                ("o2a", [128, S]),
                ("o2b", [128, S]),
                ("ex0", [128, 8 * 1024]),
                ("sc0", [128, 8 * 1024]),
                ("po0", [65, 1024]),
                ("poc0", [65, 1024]),
                ("rbr0", [64, 1024]),
            )
        }

    with tile.TileContext(nc) as tc, ExitStack() as ctx:
        const = ctx.enter_context(tc.tile_pool(name="const", bufs=1))
        # [p, lo/hi, g%4, r]
        wqt = [
            const.tile([128, 2, 4, 384], F8, tag=f"wq{k}", name=f"wq_sb{k}")
            for k in range(4)
        ]

        def wq_hi_pair(g2, m):
            # hi weights for the k-tile pair (g2, g2+1); [128, 2, 128]
            k, lj = divmod(g2, 4)
            return wqt[k][:, 1, lj : lj + 2, 128 * m : 128 * m + 128]

        def wq_cross(g, m):
            # (lo, hi) weights of one k-tile; [128, 2, 128]
            k, lj = divmod(g, 4)
            return wqt[k][:, 0:2, lj, 128 * m : 128 * m + 128]

        F32R = mybir.dt.float32r
        # [p, lo/hi, u, e]
        wo8 = const.tile([128, 2, 2, 2048], F8, tag="wo")
        rqq_sb = const.tile([128, 4096], F16, tag="rqq")
        tri_sb = const.tile([128, 256], F16, tag="tri")
        idf_sb = const.tile([128, 64], FP, tag="idf")
        ident64 = idf_sb[64:128, :].bitcast(F32R)

        qsw = ctx.enter_context(tc.tile_pool(name="qsw", bufs=1))
        q01s = qsw.tile([128, S], F16, tag="q01s")
        q23s = qsw.tile([128, S], F16, tag="q23s")
        ks = qsw.tile([128, S], F16, tag="ks")
        v_sb = qsw.tile([128, 16 * 65], F16, tag="v")
        o2a = qsw.tile([128, S], F16, tag="o2a")
        o2b = qsw.tile([128, S], F16, tag="o2b")
        # [p, hi/lo, a/b, s] fp8 split of the (16x-scaled) attention output
        o28 = qsw.tile([128, 2, 2, S], F8, tag="o28")
        v65 = v_sb.rearrange("p (j c) -> p j c", c=65)

        # attention pools (live through the whole kernel)
        expool = ctx.enter_context(tc.tile_pool(name="ex", bufs=4))
        rspool = ctx.enter_context(tc.tile_pool(name="rs", bufs=2))
        rbpool = ctx.enter_context(tc.tile_pool(name="rb", bufs=2))
        nmpool = ctx.enter_context(tc.tile_pool(name="nm", bufs=2))
        # attention psum pools are created after quarter 0 releases its
        # 3-bank ring (right side of the arena; release is LIFO per side)
        phaseA = ExitStack()
        psS = psO = None

        # projection-phase pools (close before the out-projection opens)
        phaseP = ExitStack()
        hpool = phaseP.enter_context(tc.tile_pool(name="hp", bufs=24))
        qraw = phaseP.enter_context(tc.tile_pool(name="qraw", bufs=1))
        q01 = qraw.tile([128, S], F16, tag="q01")
        q23 = qraw.tile([128, S], F16, tag="q23")
        kv = qraw.tile([128, S], F16, tag="kv")  # rows 0:64 = k (rope input)
        vraw = qraw.tile([128, S], FP, tag="vraw")  # rows 64:128 = v, f32
        scpool = phaseP.enter_context(tc.tile_pool(name="sc", bufs=4))
        phase0 = ExitStack()
        psA0 = phase0.enter_context(tc.tile_pool(name="psA0", bufs=1, space="PSUM"))
        psA = None

        SWAP_MASK = [i ^ 1 for i in range(32)]

        def rope_quarter(dst, raw, p, costab, sintab, q, nm):
            # dst = raw * cos + pairswap(raw) * sin' on [0:p, 512q:512q+512]
            cs = ds(512 * q, 512)
            sw = scpool.tile([128, 512], F16, tag="sc", name=f"sw_{nm}{q}")
            nc.vector.stream_shuffle(sw[0:p, :], raw[0:p, cs], SWAP_MASK)
            t0 = scpool.tile([128, 512], F16, tag="sc", name=f"t0_{nm}{q}")
            nc.vector.tensor_mul(t0[0:p, :], raw[0:p, cs], costab[0:p, cs])
            nc.vector.tensor_mul(sw[0:p, :], sw[0:p, :], sintab[0:p, cs])
            nc.vector.tensor_add(dst[0:p, cs], t0[0:p, :], sw[0:p, :])

        # global DMA plan: one ordered stream of large transfers.  hT comes as
        # 2048-column quads ([128, 4, 512] strided, >=512B contiguous runs),
        # hi half before lo half so the hi*hi matmuls can start early.
        hT_hi_r = hT_hi.rearrange("p (g s) -> p g s", s=2048)
        hT_lo_r = hT_lo.rearrange("p (g s) -> p g s", s=2048)
        rq_r = rqq.rearrange("p (h s) -> p h s", s=2048)
        hq_tiles = {}

        def dma_hq(q, t, which, half=None, eng=None):
            # quad t of quarter q, which: 0 hi / 1 lo
            if half is None:
                gs, n = 4 * t, 4
            else:
                gs, n = 4 * t + 2 * half, 2
            tile = hq_tiles.get((q, t))
            if tile is None:
                tile = hpool.tile([128, 2, 4, 512], F8, tag="hc", name=f"hq_{q}_{t}")
                hq_tiles[(q, t)] = tile
            src_r = hT_hi_r if which == 0 else hT_lo_r
            (eng or nc.sync).dma_start(
                tile[:, which, gs - 4 * t : gs - 4 * t + n, :],
                src_r[:, gs : gs + n, ds(512 * q, 512)],
            )

        def hq_hi_pair(q, g2):
            k, lj = divmod(g2, 4)
            return hq_tiles[(q, k)][:, 0, lj : lj + 2, :]

        def hq_cross(q, g):
            k, lj = divmod(g, 4)
            return hq_tiles[(q, k)][:, 0:2, lj, :]

        def dma_wq(k, which, eng=None):
            # which: 0 lo / 1 hi
            src = (wq_lo if which == 0 else wq_hi)[:, ds(1536 * k, 1536)]
            (eng or nc.sync).dma_start(
                wqt[k][:, which], src.rearrange("p (g r) -> p g r", r=384)
            )

        rqsb_r = rqq_sb.rearrange("p (h s) -> p h s", s=2048)

        def dma_rq(q):
            nc.sync.dma_start(
                rqsb_r[:, :, ds(512 * q, 512)], rq_r[:, :, ds(512 * q, 512)]
            )

        def dma_wo(which):
            src = (wo_lo if which == 0 else wo_hi)
            nc.sync.dma_start(
                wo8[:, which], src.rearrange("p (u e) -> p u e", e=2048)
            )

        def dma_wq0h(half):
            # startup split: pairs (0,1) land before (2,3)
            src = wq_hi[:, ds(768 * half, 768)]
            nc.sync.dma_start(
                wqt[0][:, 1, 2 * half : 2 * half + 2, :],
                src.rearrange("p (g r) -> p g r", r=384),
            )

        # quarter-0 feed is split across the SP and ACT DGE queues (same-
        # queue transfers serialize in the DMA model; ACT is idle until the
        # first evacuation): hq-hi + wq-lo on SP, wq-hi + hq-lo on ACT, so
        # each quad's supply is ~1.3us/queue vs 1.9us of PE work.
        dma_plan = [
            ("wq0ha", lambda: dma_wq0h(0)),
            ("hq00ha", lambda: dma_hq(0, 0, 0, 0)),
            ("wq0hb", lambda: dma_wq0h(1)),
            ("hq00hb", lambda: dma_hq(0, 0, 0, 1)),
            ("hq00l", lambda: dma_hq(0, 0, 1, eng=nc.gpsimd)),
            ("wq0l", lambda: dma_wq(0, 0)),
            ("wq1h", lambda: dma_wq(1, 1, eng=nc.gpsimd)),
            ("hq01h", lambda: dma_hq(0, 1, 0)),
            ("hq01l", lambda: dma_hq(0, 1, 1, eng=nc.gpsimd)),
            ("wq1l", lambda: dma_wq(1, 0)),
            ("wq2h", lambda: dma_wq(2, 1, eng=nc.gpsimd)),
            ("hq02h", lambda: dma_hq(0, 2, 0)),
            ("hq02l", lambda: dma_hq(0, 2, 1, eng=nc.gpsimd)),
            ("wq2l", lambda: dma_wq(2, 0)),
            ("wq3h", lambda: dma_wq(3, 1, eng=nc.gpsimd)),
            ("hq03h", lambda: dma_hq(0, 3, 0)),
            ("hq03l", lambda: dma_hq(0, 3, 1, eng=nc.gpsimd)),
            ("wq3l", lambda: dma_wq(3, 0)),
            ("tri", lambda: (nc.sync.dma_start(tri_sb[:], tri),
                             nc.sync.dma_start(idf_sb[:].bitcast(F32R), idf))),
            ("rq0", lambda: dma_rq(0)),
            ("hq10", lambda: (dma_hq(1, 0, 0), dma_hq(1, 0, 1))),
            ("hq11", lambda: (dma_hq(1, 1, 0), dma_hq(1, 1, 1))),
            ("hq12", lambda: (dma_hq(1, 2, 0), dma_hq(1, 2, 1))),
            ("rq1", lambda: dma_rq(1)),
            ("hq13", lambda: (dma_hq(1, 3, 0), dma_hq(1, 3, 1))),
            ("hq20", lambda: (dma_hq(2, 0, 0), dma_hq(2, 0, 1))),
            ("wo0", lambda: dma_wo(1)),
            ("hq21", lambda: (dma_hq(2, 1, 0), dma_hq(2, 1, 1))),
            ("hq22", lambda: (dma_hq(2, 2, 0), dma_hq(2, 2, 1))),
            ("wo1", lambda: dma_wo(0)),
            ("rq2", lambda: dma_rq(2)),
            ("hq23", lambda: (dma_hq(2, 3, 0), dma_hq(2, 3, 1))),
            ("hq30", lambda: (dma_hq(3, 0, 0), dma_hq(3, 0, 1))),
            ("hq31", lambda: (dma_hq(3, 1, 0), dma_hq(3, 1, 1))),
            ("hq32", lambda: (dma_hq(3, 2, 0), dma_hq(3, 2, 1))),
            ("rq3", lambda: dma_rq(3)),
            ("hq33", lambda: (dma_hq(3, 3, 0), dma_hq(3, 3, 1))),
        ]
        plan_pos = {key: idx for idx, (key, _) in enumerate(dma_plan)}
        feed_state = {"next": 0}

        def feed_until(key):
            stop = plan_pos[key] + 1
            while feed_state["next"] < stop:
                dma_plan[feed_state["next"]][1]()
                feed_state["next"] += 1

        def proj_quad(q, t, ms, pts, started, last=()):
            # all fp8 matmuls of quad t for the m-passes in ms.
            # last: set of m whose final matmul gets stop=True.
            # jj/lg outer so the DMA pieces are consumed in arrival order.
            for jj in range(2):
                g2 = 4 * t + 2 * jj
                for m in ms:
                    nc.tensor.matmul(
                        pts[m][:], wq_hi_pair(g2, m), hq_hi_pair(q, g2),
                        start=(m not in started), stop=False, perf_mode=DR,
                    )
                    started.add(m)
            for lg in range(4):
                g = 4 * t + lg
                for m in ms:
                    nc.tensor.matmul(
                        pts[m][:], wq_cross(g, m), hq_cross(q, g),
                        start=False, stop=(m in last and lg == 3), perf_mode=DR,
                    )

        def emit_vtrans(q, vtp):
            # v transposes for quarter q: f32r through the projection psum
            # ring slot, then one f16 evacuation into v65
            vt = vtp.tile([128, 512], FP, tag=vtp_tag(vtp), name=f"vt_{q}")
            for jj in range(4):
                j = 4 * q + jj
                nc.tensor.transpose(
                    vt[:, ds(64 * jj, 64)].bitcast(F32R),
                    vraw[64:128, ds(128 * j, 128)].bitcast(F32R),
                    ident64,
                )
            nc.vector.tensor_copy(
                v65[:, 4 * q : 4 * q + 4, 0:64],
                vt[:, 0:256].rearrange("p (jj c) -> p jj c", c=64),
            )
            nc.vector.tensor_copy(
                v65[:, 4 * q : 4 * q + 4, 64:65],
                inv16[:, 0:1][:, None, :].to_broadcast([128, 4, 1]),
            )

        def vtp_tag(pool):
            return "p0" if pool is psA0 else "pj"

        def evac_kv(pts_m, q):
            cs = ds(512 * q, 512)
            nc.scalar.copy(kv[0:64, cs], pts_m[0:64, :])
            # v psum is 64x; descale to true scale on the way out
            nc.scalar.mul(vraw[64:128, cs].bitcast(F32R), pts_m[64:128, :], 1.0 / 64.0)

        def gen_proj_quarter0():
            # quarter 0 runs all three m-passes interleaved at the pace the
            # DMA stream can sustain (the front is inherently supply-bound)
            pts = [
                psA0.tile([128, 512], FP, tag=f"p{m}", name=f"pj0_{m}")
                for m in range(3)
            ]
            started = set()
            feed_until("hq00hb")
            for t in range(3):
                # hi*hi needs this quad's hi pieces; cross needs lo too
                feed_until(["wq0l", "wq1l", "wq2l"][t])
                proj_quad(0, t, (0, 1, 2), pts, started)
                feed_until(["wq3l", "tri", "rq0"][t])
                yield 1280
            # quad 3 m-serially, kv first: rope-k (which gates the first
            # attention scores) starts earliest
            cos_t, sin_t = rqq_sb[:, 0:2048], rqq_sb[:, 2048:4096]
            feed_until("wq3l")
            for m, dst, dsts, p, nm in (
                (2, kv, ks, 64, "k"),
                (0, q01, q01s, 128, "q01"),
                (1, q23, q23s, 128, "q23"),
            ):
                proj_quad(0, 3, (m,), pts, started, last={m})
                if m == 2:
                    evac_kv(pts[m], 0)
                else:
                    nc.scalar.copy(dst[:, 0:512], pts[m][:])
                rope_quarter(dsts, dst, p, cos_t, sin_t, 0, nm)
                if m == 2:
                    nc.sync.dma_start(ks[64:128, 0:512], ks[0:64, 0:512])
                feed_until("hq10")
                yield 940
            emit_vtrans(0, psA0)
            yield 80
            feed_until("hq11")
            yield 120

        def gen_proj_quarter(q):
            feed_until(f"hq{q}1")
            pts = {}
            for m in (0, 1):
                pts[m] = psA.tile([128, 512], FP, tag="pj", name=f"pj_{q}_{m}")
            started = set()
            for t in range(4):
                if t >= 2:
                    feed_until(f"hq{q}{min(t + 1, 3)}")
                proj_quad(q, t, (0, 1), pts, started,
                          last=({0, 1} if t == 3 else ()))
                yield 1280
            cs = ds(512 * q, 512)
            nc.scalar.copy(q01[:, cs], pts[0][:])
            rope_quarter(q01s, q01, 128, rqq_sb[:, 0:2048], rqq_sb[:, 2048:4096], q, "q01")
            yield 300
            nc.scalar.copy(q23[:, cs], pts[1][:])
            rope_quarter(q23s, q23, 128, rqq_sb[:, 0:2048], rqq_sb[:, 2048:4096], q, "q23")
            yield 300
            pt2 = psA.tile([128, 512], FP, tag="pj", name=f"pj_{q}_2")
            started2 = set()
            for t in range(4):
                proj_quad(q, t, (2,), pts={2: pt2}, started=started2,
                          last=({2} if t == 3 else ()))
                # deep-prefetch the next quarter while the psum ring is
                # the only DMA consumer
                if q < 3:
                    feed_until(f"hq{q + 1}{min(t, 3)}")
                yield 640
            evac_kv(pt2, q)
            rope_quarter(ks, kv, 64, rqq_sb[:, 0:2048], rqq_sb[:, 2048:4096], q, "k")
            yield 300
            # duplicate rotated k at partitions 64-127 (odd heads' score
            # matmuls read lhsT/rhs both at base 64)
            nc.sync.dma_start(ks[64:128, cs], ks[0:64, cs])
            emit_vtrans(q, psA)
            if q < 3:
                feed_until(f"hq{q + 1}1")
            yield 120

        def gen_attention_chunk(c):
            nj = 4 * c + 4
            for hp in range(2):
                po = psO.tile([65, 1024], FP, tag="po", name=f"po_{c}_{hp}")

                def emit_scores(j):
                    r = j - 4 * c  # >= 0 on diagonal blocks
                    off = 128 * r if r >= 0 else 0
                    ps = psS.tile([128, 1024], FP, tag="ps", name=f"ps_{c}_{hp}_{j}")
                    for hh in range(2):
                        h = 2 * hp + hh
                        qt = q01s if h < 2 else q23s
                        base = 64 * (h % 2)
                        nc.tensor.matmul(
                            ps[:, ds(512 * hh + off, 512 - off)],
                            ks[base : base + 64, ds(128 * j, 128)],
                            qt[base : base + 64, ds(512 * c + off, 512 - off)],
                        )
                    return ps, off, r >= 0

                def emit_expav(j, ps, off, diag):
                    # exp(s/8 - 2): 1/8 is the scores 1/sqrt(hd); softmax is
                    # shift-invariant and the bias keeps the f16 sums and the
                    # denominator reciprocal in range
                    ex = expool.tile([128, 1024], F16, tag="ex", name=f"ex_{c}_{hp}_{j}")
                    if not diag:
                        nc.scalar.activation(ex[:], ps[:], EXP, bias=-1.0, scale=0.125)
                    else:
                        w = 512 - off
                        psv = ps.rearrange("p (h w) -> p h w", w=512)[:, :, ds(off, w)]
                        exv = ex.rearrange("p (h w) -> p h w", w=512)[:, :, ds(off, w)]
                        nc.scalar.activation(exv, psv, EXP, bias=-1.0, scale=0.125)
                        exd = ex.rearrange("p (h w) -> p h w", w=512)[:, :, ds(off, 128)]
                        nc.vector.tensor_mul(
                            exd,
                            exd,
                            tri_sb[:, 0:128][:, None, :].to_broadcast([128, 2, 128]),
                        )
                    if debug and c == 0:
                        sl = ds(1024 * (4 * hp + j), 1024)
                        nc.sync.dma_start(dbg["ex0"][:, sl], ex[:])
                        sc16 = expool.tile(
                            [128, 1024], F16, tag="ex", name=f"scd_{hp}_{j}"
                        )
                        nc.vector.tensor_copy(sc16[:], ps[:])
                        nc.sync.dma_start(dbg["sc0"][:, sl], sc16[:])
                    for hh in range(2):
                        nc.tensor.matmul(
                            po[0:65, ds(512 * hh + off, 512 - off)],
                            v_sb[:, ds(65 * j, 65)],
                            ex[:, ds(512 * hh + off, 512 - off)],
                            start=(j == 0),
                            stop=(j == nj - 1),
                            skip_group_check=True,
                        )

                # one-j lookahead: scores(j+1) land on the PE between
                # scores(j) and av(j) so the exp never stalls the PE
                def jcost(j):
                    # PE ns of one scores OR av pair at this block's trim
                    r = j - 4 * c
                    off = 128 * r if r >= 0 else 0
                    return int((512 - off) * 0.833)

                prev = emit_scores(0)
                for j in range(1, nj):
                    cur = emit_scores(j)
                    emit_expav(j - 1, *prev)
                    prev = cur
                    yield jcost(j) + jcost(j - 1)
                emit_expav(nj - 1, *prev)
                yield jcost(nj - 1)
                # evacuate the accumulator so the bank frees for the next
                # head pair, then normalize: reciprocal of the sums row,
                # broadcast back into the evacuated po bank, then per half:
                # f16 o2 piece -> fp8 hi (ACT cast) + fp8 lo (DVE subtract)
                poc = rspool.tile([65, 1024], F16, tag="rs", name=f"poc_{c}_{hp}")
                if debug and c == 0 and hp == 0:
                    pod = rspool.tile([65, 1024], F16, tag="pod", name="pod")
                    nc.vector.tensor_copy(pod[:], po[:])
                    nc.sync.dma_start(dbg["po0"], pod[:])
                nc.scalar.copy(poc[:, 512:1024], po[:, 512:1024])
                nc.vector.tensor_copy(poc[:, 0:512], po[:, 0:512])
                rbr = rbpool.tile([64, 1024], F16, tag="rbr", name=f"rbr_{c}_{hp}")
                dsttile = o2a if hp == 0 else o2b
                nm = nmpool.tile([64, 512], F16, tag="nm", name=f"nm_{c}_{hp}")
                nm8 = nmpool.tile([64, 2, 512], F8, tag="nm8", name=f"nm8_{c}_{hp}")
                cs = ds(512 * c, 512)
                for half in (1, 0):
                    hs = ds(512 * half, 512)
                    nc.tensor.matmul(
                        po[0:64, hs], tri_sb[64:65, 64:128], poc[64:65, hs],
                        start=True, stop=True,
                    )
                    # the fp8 hi/lo split runs on the (otherwise idle) Pool
                    # engine, except the last chunk where ACT/DVE are free
                    # and Pool's ~1us/op latency would stretch the tail; the
                    # upper-half DMAs dispatch as soon as each piece exists
                    if c == 3:
                        cast_f = nc.scalar.copy
                        sub_e = nc.gpsimd if half == 1 else nc.vector
                    else:
                        cast_f, sub_e = nc.gpsimd.tensor_copy, nc.gpsimd
                    with nc.allow_low_precision(reason="softmax denom recip f16"):
                        nc.vector.reciprocal(rbr[0:64, hs], po[0:64, hs])
                        if half == 0:
                            piece = dsttile[0:64, cs]
                            nc.vector.tensor_mul(piece, poc[0:64, hs], rbr[0:64, hs])
                            hi8 = o28[0:64, 0, hp, cs]
                            cast_f(hi8, piece)
                            sub_e.tensor_sub(o28[0:64, 1, hp, cs], piece, hi8)
                        else:
                            nc.vector.tensor_mul(nm[0:64, :], poc[0:64, hs], rbr[0:64, hs])
                            cast_f(nm8[:, 0, :], nm[0:64, :])
                            nc.sync.dma_start(o28[64:128, 0, hp, cs], nm8[:, 0, :])
                            sub_e.tensor_sub(nm8[:, 1, :], nm[0:64, :], nm8[:, 0, :])
                            nc.sync.dma_start(o28[64:128, 1, hp, cs], nm8[:, 1, :])
                if debug and c == 0 and hp == 0:
                    nc.sync.dma_start(dbg["poc0"], poc[:])
                    nc.sync.dma_start(dbg["rbr0"], rbr[0:64, :])
                yield 100

        post = {}

        def open_post_pools():
            post["ost"] = ctx.enter_context(tc.tile_pool(name="ost", bufs=6))
            post["psP"] = ctx.enter_context(tc.tile_pool(name="psP", bufs=2, space="PSUM"))

        def gen_outproj_chunk(c, tail=False, pskey="psP", bs=range(4)):
            for b in bs:
                for n2 in range(2):  # pairs of 512-wide e-slices -> one DMA
                    st = post["ost"].tile(
                        [128, 1024], F16, tag="st", name=f"st_{c}_{b}_{n2}"
                    )
                    for nn in range(2):
                        n = 2 * n2 + nn
                        pp = post[pskey].tile(
                            [128, 512], FP, tag="pp", name=f"pp_{c}_{b}_{n}"
                        )
                        cs128 = ds(512 * c + 128 * b, 128)
                        ns = ds(512 * n, 512)
                        nc.tensor.matmul(
                            pp[:], o28[:, 0, 0:2, cs128], wo8[:, 1, 0:2, ns],
                            start=True, stop=False, perf_mode=DR,
                        )
                        nc.tensor.matmul(
                            pp[:], o28[:, 0:2, 0, cs128], wo8[:, 0:2, 0, ns],
                            start=False, stop=False, perf_mode=DR,
                        )
                        nc.tensor.matmul(
                            pp[:], o28[:, 0:2, 1, cs128], wo8[:, 0:2, 1, ns],
                            start=False, stop=True, perf_mode=DR,
                        )
                        # in the pure-PE tail alternate evacuation engines so
                        # the psum ring keeps pace with the matmuls
                        if tail and nn == 1:
                            nc.scalar.copy(st[:, ds(512, 512)], pp[:])
                        else:
                            nc.vector.tensor_copy(st[:, ds(512 * nn, 512)], pp[:])
                        yield 320
                    nc.sync.dma_start(
                        out[ds(128 * (4 * c + b), 128), ds(1024 * n2, 1024)], st[:]
                    )

        def chain(*gens):
            for g in gens:
                yield from g

        def closer():
            phaseP.close()
            open_post_pools()
            return
            yield  # pragma: no cover

        def weave(ga, gb, wa=1.0, wb=1.0):
            # proportional-progress interleave of two emission streams:
            # step the stream with the smaller fraction-complete so a short
            # filler spreads across the whole window instead of front-loading
            ta = tb = 0.0
            da = db = False
            while not (da and db):
                if db or (not da and ta / wa <= tb / wb):
                    try:
                        ta += next(ga)
                    except StopIteration:
                        da = True
                else:
                    try:
                        tb += next(gb)
                    except StopIteration:
                        db = True

        def run(g):
            for _ in g:
                pass

        # ---- pipeline: P0 [P1|A0] [P2|A1] [P3,close,O0|A2] [O1,O2|A3] O3 --
        run(gen_proj_quarter0())
        phase0.close()
        psA = phaseP.enter_context(tc.tile_pool(name="psA", bufs=2, space="PSUM"))
        psS = phaseA.enter_context(
            tc.tile_pool(name="psS", bufs=2, space="PSUM", side="right")
        )
        psO = phaseA.enter_context(
            tc.tile_pool(name="psO", bufs=1, space="PSUM", side="right")
        )
        weave(gen_proj_quarter(1), gen_attention_chunk(0))
        weave(gen_proj_quarter(2), gen_attention_chunk(1))
        weave(
            chain(gen_proj_quarter(3), closer(), gen_outproj_chunk(0)),
            gen_attention_chunk(2),
            wa=13.5,
            wb=16.6,
        )
        weave(
            chain(gen_outproj_chunk(1), gen_outproj_chunk(2, bs=range(3))),
            gen_attention_chunk(3),
            wa=8.9,
            wb=22.1,
        )
        # attention psum freed -> deep out-proj ring; the O2 remainder hides
        # the last normalize chain before O3 starts
        phaseA.close()
        post["psP2"] = ctx.enter_context(
            tc.tile_pool(name="psP2", bufs=4, space="PSUM", side="right")
        )
        run(gen_outproj_chunk(2, tail=True, pskey="psP2", bs=range(3, 4)))
        # PE-warming matmuls: chunk 3's out-projection can't start until its
        # normalize -> fp8 split -> upper-half DMA chain completes (~4us).
        # Discarded fp8 matmuls keep the array busy so the p-state ramp stays
        # hot and the tail runs at full clock the moment o28 is ready.
        N_WARM = 16
        if N_WARM:
            psW = ctx.enter_context(
                tc.tile_pool(name="psW", bufs=1, space="PSUM", side="right")
            )
            warm = psW.tile([128, 512], FP, tag="warm", name="warm")
            for _ in range(N_WARM):
                nc.tensor.matmul(
                    warm[:, 0:256], wo8[:, 0, 0, 0:128], wo8[:, 0, 0, 256:512],
                    start=True, stop=True,
                )
        run(gen_outproj_chunk(3, tail=True, pskey="psP2"))
        if debug:
            for name, tile in (
                ("q01s", q01s), ("q23s", q23s), ("ks", ks),
                ("v65", v_sb), ("o2a", o2a), ("o2b", o2b),
            ):
                nc.sync.dma_start(dbg[name], tile[:])

    nc.compile()
    return nc


def get_module(debug=False):
    key = ("nc", debug)
    if key not in _CACHE:
        _CACHE[key] = _build_module(debug=debug)
    return _CACHE[key]


def _pack16(x):
    # [16*128, N] -> [128, 16*N] with [p, N*g + n] = x[128*g + p, n]
    n = x.shape[1]
    return (
        np.ascontiguousarray(x.reshape(16, 128, n).transpose(1, 0, 2)).reshape(128, 16 * n)
    )


def _split8(x):
    # f32 -> (hi, lo) e4m3 with x ~= hi + lo
    hi = x.astype(E4)
    lo = (x - hi.astype(np.float32)).astype(E4)
    return hi, lo


def prep_inputs(hidden_states, freqs_cis, wqkv, wo):
    h = np.asarray(hidden_states, dtype=np.float32)[0]  # [S, D]
    fc = np.asarray(freqs_cis, dtype=np.float32)  # [S, 32, 2]
    wqkv = np.asarray(wqkv, dtype=np.float32)  # [3072, D]
    wo = np.asarray(wo, dtype=np.float32)  # [D, D]

    hT_f = _pack16(np.ascontiguousarray(h.T))  # [128, 16*S] f32
    hT_hi, hT_lo = _split8(hT_f)

    cos = fc[:, :, 0]  # [S, 32]
    sin = fc[:, :, 1]
    cos_ext = np.repeat(cos, 2, axis=1).T  # [64, S]
    sgn = np.where(np.arange(HD) % 2 == 0, -1.0, 1.0).astype(np.float32)[:, None]
    sin_ext = np.repeat(sin, 2, axis=1).T * sgn  # sin'[d, s]
    rqq_np = (
        np.concatenate([np.tile(cos_ext, (2, 1)), np.tile(sin_ext, (2, 1))], axis=1)
        / 32.0
    ).astype(np.float16)  # [128, 4096], 1/32 scale
    idf_np = np.zeros((128, 64), dtype=np.float32)
    idf_np[64:128] = np.eye(64, dtype=np.float32)
    tri_np = np.concatenate(
        [
            (np.arange(128)[:, None] <= np.arange(128)[None, :]).astype(np.float16),
            np.eye(128, dtype=np.float16),
        ],
        axis=1,
    )  # [128, 256]: triangle | identity

    in_maps = []
    for i in range(NCORES):
        wl = np.concatenate(
            [
                wqkv[256 * i : 256 * i + 256] * 32.0,
                wqkv[D + 64 * i : D + 64 * i + 64] * 32.0,
                wqkv[D + KV_SIZE + 64 * i : D + KV_SIZE + 64 * i + 64] * 64.0,
            ],
            axis=0,
        )  # [384, D], e4m3-friendly scales
        wq_f = _pack16(np.ascontiguousarray(wl.T))  # [128, 16*384] f32
        wq_hi, wq_lo = _split8(wq_f)
        woT = np.ascontiguousarray(wo[:, 256 * i : 256 * i + 256].T) * 64.0  # [256, D]
        wo_f = np.ascontiguousarray(woT.reshape(2, 128, D).transpose(1, 0, 2)).reshape(
            128, 2 * D
        )
        wo_hi, wo_lo = _split8(wo_f)
        in_maps.append(
            {
                "hT_hi": hT_hi,
                "hT_lo": hT_lo,
                "wq_hi": wq_hi,
                "wq_lo": wq_lo,
                "wo_hi": wo_hi,
                "wo_lo": wo_lo,
                "rqq": rqq_np,
                "tri": tri_np,
                "idf": idf_np,
            }
        )
    return in_maps


def run_on_hw(in_maps, trace=False, **kw):
    from concourse.bass_utils import run_bass_kernel_spmd

    nc = get_module()
    return run_bass_kernel_spmd(nc, in_maps, list(range(NCORES)), trace=trace, **kw)


def kernel(hidden_states, freqs_cis, wqkv, wo):
    in_maps = prep_inputs(hidden_states, freqs_cis, wqkv, wo)
    res = run_on_hw(in_maps)
    acc = np.zeros((S, D), dtype=np.float64)
    for r in res.results:
        acc += np.asarray(r["out"], dtype=np.float64)
    return (acc / OUT_DESCALE).astype(np.float32).reshape(1, S, D)
